# revision 27
# baseline (speedup 1.0000x reference)
"""EntropyGuidance Trainium2 kernel, fp16-I/O Act-roofline variant.

Each core handles 2 samples (B=16 over 8 cores), each sample packed as
[128 partitions = 64 channels x 2 HW-halves (p = 2c+h), 8192 free].

The Activation engine is the roofline here (~31us of exp work that no
other engine can run), so the schedule keeps Act 100% fed:
  - fp16 on both DMA directions (host casts): loads ride Pool/SWDGE
    (text) + SP/HWDGE (vis), stores SP/HWDGE; DMA busy ~35us.
  - PE is warmed with ~8 wide dummy matmuls before the first transpose
    so the p-state ramp never doubles transpose time mid-pipeline.
  - sample 0 leads with two 512-col groups so the first exp starts
    ~1us earlier; the exp->xe->transpose PSUM round-trip is kept under
    2x the exp stage time (2 tv buffers).
  - phase-2 Act ops are emitted AFTER the next sample's first exp so a
    blocked Ln never head-of-line blocks the exp stream.
  - out = vis + g*text is split DVE (tensor_scalar_mul 4x + tensor_add
    2x, 0.78 ns/col) / Pool (one-op scalar_tensor_tensor, 1.39 ns/col)
    so the tail after g1 shrinks and DVE never starves the xe products.
"""

import sys

sys.path.insert(0, "/opt/trn_rl_repo")

import math
from contextlib import ExitStack

import numpy as np

import concourse.bacc as bacc
import concourse.tile as tile
from concourse import mybir
from concourse.bass_utils import run_bass_kernel_spmd
from concourse.masks import make_identity

if not hasattr(bacc, "_orig_get_act_tables"):
    bacc._orig_get_act_tables = bacc.get_activation_tables


def _lnexp_only_tables(module_arch):
    tabs = bacc._orig_get_act_tables(module_arch)
    return {
        name: (funcs if name == "natural_log_exp_and_others" else set())
        for name, funcs in tabs.items()
    }


bacc.get_activation_tables = _lnexp_only_tables

F32 = mybir.dt.float32
FP16 = mybir.dt.float16
AF = mybir.ActivationFunctionType
ALU = mybir.AluOpType

B, C, H, W = 16, 64, 128, 128
HW = H * W                      # 16384
HH = HW // 2                    # 8192 per half
NCORES = 8
P = 128                         # partitions = 64 channels x 2 halves
EPS = 1e-9

# per-sample load pieces along the 8192 free axis
PIECES_S = [
    [(0, 512), (512, 512), (1024, 1024), (2048, 2048), (4096, 2048),
     (6144, 2048)],
    [(0, 2048), (2048, 2048), (4096, 2048), (6144, 2048)],
]
# transpose/exp groups (off, width); J blocks per group = width/128.
# sample 1 ends with two 512 groups so the last exp -> J/T-stop chain
# (which gates g1 and the tail stores) is as short as possible
GROUPS_S = [
    [(0, 512), (512, 512)] + [(1024 + g * 1024, 1024) for g in range(7)],
    [(g * 1024, 1024) for g in range(7)] + [(7168, 512), (7680, 512)],
]
# madd work split: (piece, col_lo, col_hi, engine)
# DVE gets 2-op (0.78/col) chunks; sample-0 chunks are 512 wide so an
# in-flight chunk never delays an xe product by more than ~0.4us.
# "pe" chunks run Dg@text + I@vis on the Tensor engine into PSUM with
# an Act PSUM->SBUF fp16 copy (both engines idle at the tail; PE
# chunks are 512 wide since a matmul output must fit one PSUM bank)
MADD_S = [
    # sample 0: 16 x 512-col chunks; "dvepool" = DVE 4x tensor_scalar
    # for g*text then the +vis add on the idle Pool engine, so each
    # group iteration absorbs ~3 chunks without out-pacing the 1.9us
    # Act stage on DVE
    [(0, 0, 512, "dve"), (1, 0, 512, "dve"),
     (2, 0, 512, "dve"), (2, 512, 1024, "dve"),
     (3, 0, 512, "dve"), (3, 512, 1024, "dve"),
     (3, 1024, 1536, "dve"), (3, 1536, 2048, "dve"),
     (4, 0, 512, "dve"), (4, 512, 1024, "dve"),
     (4, 1024, 1536, "dve"), (4, 1536, 2048, "dvepool"),
     (5, 0, 512, "dvepool"), (5, 512, 1024, "dvepool"),
     (5, 1024, 1536, "dvepool"), (5, 1536, 2048, "dvepool")],
    # sample 1 (tail): DVE 5120 cols, PE+Act 3072 cols in parallel
    [(0, 0, 1024, "dve"), (0, 1024, 2048, "dve"),
     (1, 0, 1024, "dve"), (1, 1024, 2048, "dve"),
     (2, 0, 1024, "dve"),
     (2, 1024, 1536, "pe"), (2, 1536, 2048, "pe"),
     (3, 0, 512, "pe"), (3, 512, 1024, "pe"),
     (3, 1024, 1536, "pe"), (3, 1536, 2048, "pe")],
]


def _grp_src(pieces, groups):
    out = []
    for off, w in groups:
        for pi, (o, pw) in enumerate(pieces):
            if o <= off and off + w <= o + pw:
                out.append((pi, off - o))
                break
        else:
            raise AssertionError((off, w))
    return out


GRP_SRC_S = [_grp_src(PIECES_S[s], GROUPS_S[s]) for s in range(2)]
NBLK = 64                       # J blocks per sample
# Ln(joint) rescale: raw J entries are ~Sv*St/HW ~ 4.5e4, so scale the act
# Ln input to ~1.4 and add ln(HW^2 / JSCALE) back via lnSv
JSCALE = 1.0 / 32768.0
LNK0 = math.log(float(HW) * float(HW) / JSCALE)


def _build_program():
    nc = bacc.Bacc()
    vis_d = nc.declare_dram_parameter("vis", [2, C, 2, HH], FP16,
                                      isOutput=False)
    text_d = nc.declare_dram_parameter("text", [2, C, 2, HH], FP16,
                                       isOutput=False)
    out_d = nc.declare_dram_parameter("out", [2, C, 2, HH], FP16,
                                      isOutput=True)

    with ExitStack() as ctx:
        tc = ctx.enter_context(tile.TileContext(nc))
        _emit(ctx, tc, vis_d, text_d, out_d)
    nc.finalize()
    return nc


def _emit(ctx: ExitStack, tc: tile.TileContext, vis_d, text_d, out_d):
    nc = tc.nc

    io = ctx.enter_context(tc.tile_pool(name="io", bufs=2))
    etvp = ctx.enter_context(tc.tile_pool(name="etv", bufs=2))
    xep = ctx.enter_context(tc.tile_pool(name="xe", bufs=2))
    outp = ctx.enter_context(tc.tile_pool(name="outp", bufs=2))
    consts = ctx.enter_context(tc.tile_pool(name="consts", bufs=1))
    small = ctx.enter_context(tc.tile_pool(name="small", bufs=2))
    # PSUM budget (8 banks): tv 3 bufs x 2 banks = 6, plus ONE bank
    # holding both samples' J/stat accumulators AND every phase-2
    # matmul output AND the warm-up target, packed as column ranges of
    # a single [P, 512] f32 tile.  The third tv buffer gives the
    # exp->xe->transpose round-trip enough slack that the Act exp
    # stream (the roofline) never stalls on it.
    tvps = ctx.enter_context(tc.tile_pool(name="tvps", bufs=3, space="PSUM"))
    jst = ctx.enter_context(tc.tile_pool(name="jst", bufs=1, space="PSUM"))

    tsb = {}   # (s, piece) -> text fp16 tile
    vsb = {}
    jtl = {}   # s -> [P, 132] f32 PSUM: J (0:128) + S_v/S_t/T (128:131)
    tvl = {}   # (s, g) -> transpose-group PSUM tile
    etl = {}   # (s, g) -> exp(group) SBUF tile
    xel = {}   # (s, g) -> t*e^t group tile

    # fp16 identity for the 128x128 PE transposes
    ident_h = consts.tile([P, P], FP16)

    def emit_loads(s, with_ident=False):
        # sample 0: text on Pool/SWDGE, vis on SP/HWDGE so both streams
        # interleave at the DMA device and group g's pair lands together.
        # sample 1: BOTH tensors on Pool/SWDGE - the SP queue issues
        # faster than SWDGE and its transfers would otherwise overtake
        # sample 0's text pieces in the DMA FIFO, starving the exp
        # stream (the critical engine) for several microseconds.
        for pi, (o, w) in enumerate(PIECES_S[s]):
            t = io.tile([P, w], FP16, tag=f"t{pi}", name=f"t{s}_{pi}")
            v = io.tile([P, w], FP16, tag=f"v{pi}", name=f"v{s}_{pi}")
            tsb[(s, pi)] = t
            vsb[(s, pi)] = v
            src_t = text_d[s, :, :, o:o + w].rearrange("c h n -> (c h) n")
            nc.gpsimd.dma_start(out=t, in_=src_t)
            src_v = vis_d[s, :, :, o:o + w].rearrange("c h n -> (c h) n")
            if s == 0:
                nc.sync.dma_start(out=v, in_=src_v)
            else:
                nc.gpsimd.dma_start(out=v, in_=src_v)
            if with_ident and pi == 0:
                # identity lands on the Pool queue right after the first
                # text piece so transposes aren't blocked behind the
                # whole load stream
                make_identity(nc, ident_h)

    def emit_transposes(s, g):
        off, w = GROUPS_S[s][g]
        pi, lo = GRP_SRC_S[s][g]
        t_src, v_src = tsb[(s, pi)], vsb[(s, pi)]
        nb = w // 128
        tv = tvps.tile([P, 2 * w], FP16, tag="tv", name=f"tv{s}_{g}")
        tvl[(s, g)] = tv
        for b in range(nb):
            c0 = lo + b * 128
            nc.tensor.transpose(tv[:, b * 128:(b + 1) * 128],
                                t_src[:, c0:c0 + 128], ident_h)
        for b in range(nb):
            c0 = lo + b * 128
            nc.tensor.transpose(tv[:, w + b * 128:w + (b + 1) * 128],
                                v_src[:, c0:c0 + 128], ident_h)

    def emit_exp_xe(s, g, split=False):
        off, w = GROUPS_S[s][g]
        tv = tvl[(s, g)]
        etv = etvp.tile([P, 2 * w], FP16, tag="etv", name=f"etv{s}_{g}")
        etl[(s, g)] = etv
        if split:
            # text half first so xe (and the J rhs) unblock earlier
            nc.scalar.activation(out=etv[:, 0:w], in_=tv[:, 0:w],
                                 func=AF.Exp)
            nc.scalar.activation(out=etv[:, w:2 * w], in_=tv[:, w:2 * w],
                                 func=AF.Exp)
        else:
            nc.scalar.activation(out=etv, in_=tv, func=AF.Exp)
        xe = xep.tile([P, w], FP16, tag="xe", name=f"xe{s}_{g}")
        xel[(s, g)] = xe
        # tensor_tensor gets the 2x DVE mode; this read also releases tv
        nc.vector.tensor_mul(xe, tv[:, 0:w], etv[:, 0:w])

    def _blk0(s, g):
        return GROUPS_S[s][g][0] // 128

    def emit_matmuls(s, g):
        off, w = GROUPS_S[s][g]
        etv, j_t = etl[(s, g)], jtl[s]
        for b in range(w // 128):
            k = _blk0(s, g) + b
            sp_f = (k == NBLK - 1)
            etT = etv[:, b * 128:(b + 1) * 128]
            evT = etv[:, w + b * 128:w + (b + 1) * 128]
            # all chains accumulate onto PSUM zeros with start=False
            nc.tensor.matmul(j_t[:, 0:128], lhsT=evT, rhs=etT,
                             start=False, stop=sp_f,
                             skip_group_check=True)
            nc.tensor.matmul(j_t[:, 128:129], lhsT=evT, rhs=ones128h,
                             start=False, stop=sp_f,
                             skip_group_check=True)
            nc.tensor.matmul(j_t[:, 129:130], lhsT=etT, rhs=ones128h,
                             start=False, stop=sp_f,
                             skip_group_check=True)

    def emit_tstats(s, g):
        # T = sum_n t*e^t column sums; emitted one group late so the PE
        # queue never blocks on the DVE xe product at the queue head
        off, w = GROUPS_S[s][g]
        xe, j_t = xel[(s, g)], jtl[s]
        for b in range(w // 128):
            k = _blk0(s, g) + b
            nc.tensor.matmul(j_t[:, 130:131],
                             lhsT=xe[:, b * 128:(b + 1) * 128],
                             rhs=ones128h, start=False,
                             stop=(k == NBLK - 1),
                             skip_group_check=True)

    def emit_phase2_stats(s):
        """negent/recips branch; independent of the J merge. Reads the
        Sv/St/T stat columns straight out of PSUM where possible."""
        parts = small.tile([P, 3], F32, tag="parts", name=f"parts{s}")
        nc.vector.tensor_copy(out=parts, in_=jtl[s][:, 128:131])
        sums_ps = jt_all[0:C, 456:459]
        nc.tensor.matmul(sums_ps, lhsT=hsum, rhs=parts, start=True,
                         stop=True)
        recips = small.tile([C, 3], F32, tag="recips", name=f"recips{s}")
        nc.vector.reciprocal(out=recips[:, 0:2], in_=sums_ps[:, 0:2])
        rst = recips[:, 1:2]
        nc.vector.tensor_scalar_mul(out=recips[:, 2:3], in0=recips[:, 0:1],
                                    scalar1=0.5)
        lnls = small.tile([C, 2], F32, tag="lnls", name=f"lnls{s}")
        nc.scalar.activation(out=lnls, in_=sums_ps[:, 0:2], func=AF.Ln)
        # fold the lt0 rescale constant ln(HW^2 / JSCALE) into lnSv so the
        # Ln of the raw joint runs on O(1) inputs (act table accuracy)
        nc.vector.tensor_scalar_add(out=lnls[:, 0:1], in0=lnls[:, 0:1],
                                    scalar1=-LNK0)
        # negent = T/St - lnSt; bias for the final Exp folds negent in:
        # biasv = -negent - (1 + HW*EPS)
        negent = small.tile([C, 1], F32, tag="negent", name=f"negent{s}")
        nc.vector.scalar_tensor_tensor(
            out=negent, in0=sums_ps[:, 2:3], scalar=rst, in1=lnls[:, 1:2],
            op0=ALU.mult, op1=ALU.subtract)
        biasv = small.tile([C, 1], F32, tag="biasv", name=f"biasv{s}")
        nc.vector.tensor_scalar(out=biasv, in0=negent, scalar1=-1.0,
                                scalar2=-(1.0 + HW * EPS), op0=ALU.mult,
                                op1=ALU.add)
        r2 = small.tile([C, 2], F32, tag="r2", name=f"r2{s}")
        nc.gpsimd.memset(r2[:, 0:1], 1.0)
        nc.gpsimd.tensor_copy(out=r2[:, 1:2], in_=lnls[:, 1:2])
        return recips, lnls, biasv, r2

    def emit_phase2_ji(s):
        """J-branch head: only needs the J matmul chain stop (emitted
        before the T-stat stop so it starts ~0.5us earlier).
        J64T[d,c] = sum_h JJ[(c,h),(d,h)]"""
        jj = small.tile([P, P], F32, tag="jj", name=f"jj{s}")
        nc.vector.tensor_copy(out=jj, in_=jtl[s][:, 0:128])
        jj_v = jj.rearrange("p (c h) -> p c h", h=2)
        j64t_ps = jt_all[0:C, 392:456]
        for h in range(2):
            nc.tensor.matmul(j64t_ps, lhsT=jj_v[:, :, h], rhs=esel[h],
                             start=(h == 0), stop=(h == 1))
        return j64t_ps

    def emit_phase2_lt0(s, j64t_ps):
        lt0 = small.tile([C, C], F32, tag="lt0", name=f"lt0{s}")
        nc.scalar.activation(out=lt0, in_=j64t_ps, func=AF.Ln,
                             scale=JSCALE)
        return lt0

    def emit_phase2_mi(s, stats, j64t_ps, lt0):
        """mi via the separable log:
        ln(HW^2*J/(Sv*St)) = ln(HW^2*Jraw) - lnSv_c - lnSt_d
        (the +EPS inside the reference log shifts values ~1e-9; dropped)
        """
        recips, lnls, biasv, r2 = stats
        rsv05 = recips[:, 2:3]
        rst = recips[:, 1:2]
        lnsv = lnls[:, 0:1]
        # PSUM->SBUF copy doubles as the rst_d row scaling
        j64t = small.tile([C, C], F32, tag="j64t_sb", name=f"j64t_sb{s}")
        nc.vector.tensor_scalar_mul(out=j64t, in0=j64t_ps, scalar1=rst)
        q = small.tile([C, C], F32, tag="q", name=f"q{s}")
        nc.gpsimd.tensor_mul(q, lt0, j64t)
        ry_ps = jt_all[0:C, 459:461]
        nc.tensor.matmul(ry_ps, lhsT=j64t, rhs=r2, start=True, stop=True)
        u1_ps = jt_all[0:C, 461:462]
        nc.tensor.matmul(u1_ps, lhsT=q, rhs=ones64[:, 0:1], start=True,
                         stop=True)
        # z1 = ry0*lnsv + ry1 ; zz = (u1 - z1) * 0.5/Sv  (two-scalar ops)
        z1 = small.tile([C, 1], F32, tag="z1", name=f"z1{s}")
        nc.vector.tensor_scalar(out=z1, in0=ry_ps[:, 0:1], scalar1=lnsv,
                                scalar2=ry_ps[:, 1:2], op0=ALU.mult,
                                op1=ALU.add)
        zz = small.tile([C, 1], F32, tag="zz", name=f"zz{s}")
        nc.vector.tensor_scalar(out=zz, in0=u1_ps, scalar1=z1,
                                scalar2=rsv05, op0=ALU.subtract,
                                op1=ALU.mult)
        mib_ps = jt_all[0:C, 462:463]
        nc.tensor.matmul(mib_ps, lhsT=ones64, rhs=zz, start=True,
                         stop=True)
        return mib_ps

    def emit_phase2_g(s, mib_ps, stats):
        biasv = stats[2]
        g64 = small.tile([C, 1], F32, tag="g64", name=f"g64{s}")
        nc.scalar.activation(out=g64, in_=mib_ps, func=AF.Exp, scale=-1.0,
                             bias=biasv)
        nc.vector.tensor_scalar_add(out=g64, in0=g64, scalar1=1.0)
        nc.vector.reciprocal(out=g64, in_=g64)
        g_ps = jt_all[:, 463:464]
        nc.tensor.matmul(g_ps, lhsT=hsumT, rhs=g64, start=True, stop=True)
        g = small.tile([P, 1], F32, tag="g", name=f"g{s}")
        nc.vector.tensor_copy(out=g, in_=g_ps)
        return g

    otl = {}

    def emit_madd(s, g, ci, dg=None):
        pi, lo, hi, eng = MADD_S[s][ci]
        o, w = PIECES_S[s][pi]
        if (s, pi) not in otl:
            otl[(s, pi)] = outp.tile([P, w], FP16, tag=f"o{s}_{pi}",
                                     name=f"o{s}_{pi}")
        ot = otl[(s, pi)]
        if eng == "dve" or eng == "dvepool":
            # g*text at 4x (f32 ptr scalar exempt from the 2-byte rule),
            # then += vis at 2x (or on the idle Pool for dvepool)
            nc.vector.tensor_scalar_mul(out=ot[:, lo:hi],
                                        in0=tsb[(s, pi)][:, lo:hi],
                                        scalar1=g)
            add_eng = nc.vector if eng == "dve" else nc.gpsimd
            add_eng.tensor_add(ot[:, lo:hi], ot[:, lo:hi],
                               vsb[(s, pi)][:, lo:hi])
        else:
            # Dg@text + I@vis accumulated in PSUM, Act copies to fp16
            pm = tvps.tile([P, hi - lo], F32, tag="tv",
                           name=f"pm{s}_{ci}")
            nc.tensor.matmul(pm, lhsT=dg, rhs=tsb[(s, pi)][:, lo:hi],
                             start=True, stop=False)
            nc.tensor.matmul(pm, lhsT=ident_h, rhs=vsb[(s, pi)][:, lo:hi],
                             start=False, stop=True)
            nc.scalar.activation(out=ot[:, lo:hi], in_=pm, func=AF.Copy)

    store_ctr = [0]

    def emit_store(s, pi, lo=None, hi=None):
        # alternate SP/Pool queues so store issue never serializes on
        # one sequencer at the tail; lo/hi store a piece sub-range so
        # tail halves stream out as soon as their chunks finish
        o, w = PIECES_S[s][pi]
        if lo is None:
            lo, hi = 0, w
        dst = out_d[s, :, :, o + lo:o + hi].rearrange("c h n -> (c h) n")
        eng = nc.sync if store_ctr[0] % 2 == 0 else nc.gpsimd
        store_ctr[0] += 1
        eng.dma_start(out=dst, in_=otl[(s, pi)][:, lo:hi])

    # ---- emission ----
    # single-bank PSUM mega-tile: j0 0:132, j1 132:264, warm 264:392,
    # j64t 392:456, sums 456:459, ry 459:461, u1 461, mib 462, gbc 463
    jt_all = jst.tile([P, 512], F32, tag="jall", name="jall")
    for s in range(2):
        jtl[s] = jt_all[:, s * 132:(s + 1) * 132]

    # DVE-built constants + PE warm-up fodder (DVE is idle at t=0)
    ones128h = consts.tile([P, 1], FP16)
    nc.vector.memset(ones128h, 1.0)
    junk = consts.tile([P, 192], FP16)
    nc.vector.memset(junk, 0.0)

    emit_loads(0, with_ident=True)

    # nudge the PE p-state ramp before the first transposes (engine
    # init means PE can't start before ~2.4us; the first piece lands
    # ~2.9us, so just a few warms to leave the lowest p-state)
    warm_ps = jt_all[0:1, 264:392]
    for i in range(3):
        nc.tensor.matmul(warm_ps, lhsT=ones128h, rhs=junk[:, 0:128],
                         start=True, stop=True)

    # phase-2 constants on the Pool queue; they are only needed ~15us in
    hsum = consts.tile([P, C], F32)
    nc.gpsimd.memset(hsum, 0.0)
    for base in (0, -1):   # fill where p - 2c + base == 0
        nc.gpsimd.affine_select(out=hsum, in_=hsum,
                                compare_op=ALU.not_equal, fill=1.0,
                                base=base, pattern=[[-2, C]],
                                channel_multiplier=1)
    esel = []
    for h in range(2):     # E_h[p, c] = 1 iff p == 2c + h
        e = consts.tile([P, C], F32, tag=f"esel{h}", name=f"esel{h}")
        nc.gpsimd.memset(e, 0.0)
        nc.gpsimd.affine_select(out=e, in_=e, compare_op=ALU.not_equal,
                                fill=1.0, base=-h, pattern=[[-2, C]],
                                channel_multiplier=1)
        esel.append(e)
    hsumT = consts.tile([C, P], F32)
    nc.gpsimd.memset(hsumT, 0.0)
    for base in (0, -1):   # fill where p - 2c + base == 0
        nc.gpsimd.affine_select(out=hsumT, in_=hsumT,
                                compare_op=ALU.not_equal, fill=1.0,
                                base=base, pattern=[[1, P]],
                                channel_multiplier=-2)
    ones64 = consts.tile([C, C], F32)
    nc.gpsimd.memset(ones64, 1.0)

    NG0 = len(GROUPS_S[0])
    NG1 = len(GROUPS_S[1])

    # sample 0, software-pipelined: transposes one group ahead of the
    # J/S matmuls, T-stat matmuls one group behind (they wait on DVE xe)
    emit_transposes(0, 0)
    emit_transposes(0, 1)
    emit_exp_xe(0, 0, split=True)
    emit_matmuls(0, 0)
    for g in range(2, NG0):
        emit_transposes(0, g)
        emit_exp_xe(0, g - 1)
        emit_matmuls(0, g - 1)
        emit_tstats(0, g - 2)
    emit_exp_xe(0, NG0 - 1)
    emit_matmuls(0, NG0 - 1)
    ji0 = emit_phase2_ji(0)       # J branch head right at J-stop
    emit_tstats(0, NG0 - 2)
    emit_tstats(0, NG0 - 1)

    emit_loads(1)

    # sample 1 pipelined; sample-0 phase 2 is emitted AFTER exp(1,0) so
    # its (dependency-blocked) Act ops never head-of-line block the exp
    # stream; its Act ops then slot between sample-1 exps
    emit_transposes(1, 0)
    emit_transposes(1, 1)
    emit_exp_xe(1, 0)
    emit_matmuls(1, 0)

    stats0 = emit_phase2_stats(0)
    lt00 = emit_phase2_lt0(0, ji0)
    mib0 = emit_phase2_mi(0, stats0, ji0, lt00)

    emit_transposes(1, 2)
    emit_exp_xe(1, 1)
    emit_matmuls(1, 1)
    emit_tstats(1, 0)

    g0 = emit_phase2_g(0, mib0, stats0)

    # s0 madd: ~3 chunks per group iteration, emitted AFTER the xe
    # product so the tv round-trip is never delayed; the dve-pair
    # chunks plus one Pool-assisted chunk keep the per-iteration DVE
    # total under the 1.9us Act stage so the exp stream keeps pace
    madd0 = list(range(len(MADD_S[0])))
    # store piece pi once its last chunk ci is done
    last_chunk = {}
    for ci, (pi, lo, hi, eng) in enumerate(MADD_S[0]):
        last_chunk[pi] = ci

    def drip_madd0(n):
        for _ in range(n):
            if not madd0:
                return
            ci = madd0.pop(0)
            emit_madd(0, g0, ci)
            pi = MADD_S[0][ci][0]
            if last_chunk[pi] == ci:
                emit_store(0, pi)

    for g in range(3, NG1):
        emit_transposes(1, g)
        emit_exp_xe(1, g - 1)
        emit_matmuls(1, g - 1)
        emit_tstats(1, g - 2)
        drip_madd0(3)
    emit_exp_xe(1, NG1 - 1)
    emit_matmuls(1, NG1 - 1)
    ji1 = emit_phase2_ji(1)       # J branch head right at J-stop
    emit_tstats(1, NG1 - 2)
    emit_tstats(1, NG1 - 1)

    # stats head (parts copy) gets DVE priority right at T-stop; the
    # remaining s0 madd chunks are emitted after the phase-2 heads so
    # they fill DVE idle slots without head-of-line blocking the chain
    stats1 = emit_phase2_stats(1)
    lt01 = emit_phase2_lt0(1, ji1)
    drip_madd0(3)
    mib1 = emit_phase2_mi(1, stats1, ji1, lt01)
    g1 = emit_phase2_g(1, mib1, stats1)
    drip_madd0(len(MADD_S[0]))

    # tail: PE+Act chunks and DVE chunks run in parallel; each 1024-col
    # half is stored the moment its chunks are written.  Dg = diag(g1)
    # feeds the PE path (4x tensor_scalar off ident)
    dg1 = small.tile([P, P], FP16, tag="dg", name="dg1")
    nc.vector.tensor_scalar_mul(out=dg1, in0=ident_h, scalar1=g1)
    for ci in (5, 6):                 # pe: p2[1024:2048]
        emit_madd(1, g1, ci, dg=dg1)
    emit_madd(1, g1, 4)               # dve: p2[0:1024]
    emit_store(1, 2, 0, 1024)
    emit_store(1, 2, 1024, 2048)
    for ci in (7, 8):                 # pe: p3[0:1024]
        emit_madd(1, g1, ci, dg=dg1)
    emit_madd(1, g1, 0)               # dve: p0[0:1024]
    emit_store(1, 3, 0, 1024)
    emit_store(1, 0, 0, 1024)
    for ci in (9, 10):                # pe: p3[1024:2048]
        emit_madd(1, g1, ci, dg=dg1)
    emit_madd(1, g1, 1)               # dve: p0[1024:2048]
    emit_store(1, 3, 1024, 2048)
    emit_store(1, 0, 1024, 2048)
    emit_madd(1, g1, 2)               # dve: p1[0:1024]
    emit_store(1, 1, 0, 1024)
    emit_madd(1, g1, 3)               # dve: p1[1024:2048]
    emit_store(1, 1, 1024, 2048)


_PROGRAM = None


def _get_program():
    global _PROGRAM
    if _PROGRAM is None:
        _PROGRAM = _build_program()
    return _PROGRAM


def kernel(vis_feat: np.ndarray, text_feat: np.ndarray) -> np.ndarray:
    nc = _get_program()
    vis = np.ascontiguousarray(vis_feat, dtype=np.float16)
    text = np.ascontiguousarray(text_feat, dtype=np.float16)
    bpc = B // NCORES
    in_maps = [
        {
            "vis": vis[i * bpc:(i + 1) * bpc].reshape(bpc, C, 2, HH),
            "text": text[i * bpc:(i + 1) * bpc].reshape(bpc, C, 2, HH),
        }
        for i in range(NCORES)
    ]
    res = run_bass_kernel_spmd(nc, in_maps, list(range(NCORES)))
    out = np.concatenate(
        [np.asarray(r["out"]).reshape(bpc, C, H, W) for r in res.results],
        axis=0)
    return out.astype(np.float32)


# revision 29
# speedup vs baseline: 1.0464x; 1.0464x over previous
"""EntropyGuidance Trainium2 kernel, fp16-I/O Act-roofline variant.

Each core handles 2 samples (B=16 over 8 cores), each sample packed as
[128 partitions = 64 channels x 2 HW-halves (p = 2c+h), 8192 free].

The Activation engine is the roofline here (~31us of exp work that no
other engine can run), so the schedule keeps Act 100% fed:
  - fp16 on both DMA directions (host casts): loads ride Pool/SWDGE
    (text) + SP/HWDGE (vis), stores SP/HWDGE; DMA busy ~35us.
  - PE is warmed with ~8 wide dummy matmuls before the first transpose
    so the p-state ramp never doubles transpose time mid-pipeline.
  - sample 0 leads with two 512-col groups so the first exp starts
    ~1us earlier; the exp->xe->transpose PSUM round-trip is kept under
    2x the exp stage time (2 tv buffers).
  - phase-2 Act ops are emitted AFTER the next sample's first exp so a
    blocked Ln never head-of-line blocks the exp stream.
  - out = vis + g*text is split DVE (tensor_scalar_mul 4x + tensor_add
    2x, 0.78 ns/col) / Pool (one-op scalar_tensor_tensor, 1.39 ns/col)
    so the tail after g1 shrinks and DVE never starves the xe products.
"""

import sys

sys.path.insert(0, "/opt/trn_rl_repo")

import math
from contextlib import ExitStack

import numpy as np

import concourse.bacc as bacc
import concourse.tile as tile
from concourse import mybir
from concourse.bass_utils import run_bass_kernel_spmd
from concourse.masks import make_identity

if not hasattr(bacc, "_orig_get_act_tables"):
    bacc._orig_get_act_tables = bacc.get_activation_tables


def _lnexp_only_tables(module_arch):
    tabs = bacc._orig_get_act_tables(module_arch)
    return {
        name: (funcs if name == "natural_log_exp_and_others" else set())
        for name, funcs in tabs.items()
    }


bacc.get_activation_tables = _lnexp_only_tables

F32 = mybir.dt.float32
FP16 = mybir.dt.float16
AF = mybir.ActivationFunctionType
ALU = mybir.AluOpType

B, C, H, W = 16, 64, 128, 128
HW = H * W                      # 16384
HH = HW // 2                    # 8192 per half
NCORES = 8
P = 128                         # partitions = 64 channels x 2 halves
EPS = 1e-9

# per-sample load pieces along the 8192 free axis
PIECES_S = [
    [(0, 512), (512, 512), (1024, 1024), (2048, 2048), (4096, 2048),
     (6144, 2048)],
    [(0, 2048), (2048, 2048), (4096, 2048), (6144, 2048)],
]
# transpose/exp groups (off, width); J blocks per group = width/128.
# sample 1 ends with two 512 groups so the last exp -> J/T-stop chain
# (which gates g1 and the tail stores) is as short as possible
GROUPS_S = [
    [(0, 512), (512, 512)] + [(1024 + g * 1024, 1024) for g in range(7)],
    [(g * 1024, 1024) for g in range(7)] + [(7168, 512), (7680, 512)],
]
# madd work split: (piece, col_lo, col_hi, engine)
# DVE gets 2-op (0.78/col) chunks; sample-0 chunks are 512 wide so an
# in-flight chunk never delays an xe product by more than ~0.4us.
# "pe" chunks run Dg@text + I@vis on the Tensor engine into PSUM with
# an Act PSUM->SBUF fp16 copy (both engines idle at the tail; PE
# chunks are 512 wide since a matmul output must fit one PSUM bank)
MADD_S = [
    # sample 0: 16 x 512-col chunks; "dvepool" = DVE 4x tensor_scalar
    # for g*text then the +vis add on the idle Pool engine, so each
    # group iteration absorbs ~3 chunks without out-pacing the 1.9us
    # Act stage on DVE
    [(0, 0, 512, "dve"), (1, 0, 512, "dve"),
     (2, 0, 512, "dve"), (2, 512, 1024, "dve"),
     (3, 0, 512, "dve"), (3, 512, 1024, "dve"),
     (3, 1024, 1536, "dve"), (3, 1536, 2048, "dve"),
     (4, 0, 512, "dvepool"), (4, 512, 1024, "dvepool"),
     (4, 1024, 1536, "dvepool"), (4, 1536, 2048, "dvepool"),
     (5, 0, 512, "dvepool"), (5, 512, 1024, "dvepool"),
     (5, 1024, 1536, "dvepool"), (5, 1536, 2048, "dvepool")],
    # sample 1 (tail): DVE 5120 cols, PE+Act 3072 cols in parallel
    [(0, 0, 1024, "dve"), (0, 1024, 2048, "dve"),
     (1, 0, 1024, "dve"), (1, 1024, 2048, "dve"),
     (2, 0, 1024, "dve"),
     (2, 1024, 1536, "pe"), (2, 1536, 2048, "pe"),
     (3, 0, 512, "pe"), (3, 512, 1024, "pe"),
     (3, 1024, 1536, "pe"), (3, 1536, 2048, "pe")],
]


def _grp_src(pieces, groups):
    out = []
    for off, w in groups:
        for pi, (o, pw) in enumerate(pieces):
            if o <= off and off + w <= o + pw:
                out.append((pi, off - o))
                break
        else:
            raise AssertionError((off, w))
    return out


GRP_SRC_S = [_grp_src(PIECES_S[s], GROUPS_S[s]) for s in range(2)]
NBLK = 64                       # J blocks per sample
# Ln(joint) rescale: raw J entries are ~Sv*St/HW ~ 4.5e4, so scale the act
# Ln input to ~1.4 and add ln(HW^2 / JSCALE) back via lnSv
JSCALE = 1.0 / 32768.0
LNK0 = math.log(float(HW) * float(HW) / JSCALE)


def _build_program():
    nc = bacc.Bacc()
    vis_d = nc.declare_dram_parameter("vis", [2, C, 2, HH], FP16,
                                      isOutput=False)
    text_d = nc.declare_dram_parameter("text", [2, C, 2, HH], FP16,
                                       isOutput=False)
    out_d = nc.declare_dram_parameter("out", [2, C, 2, HH], FP16,
                                      isOutput=True)

    with ExitStack() as ctx:
        tc = ctx.enter_context(tile.TileContext(nc))
        _emit(ctx, tc, vis_d, text_d, out_d)
    nc.finalize()
    return nc


def _emit(ctx: ExitStack, tc: tile.TileContext, vis_d, text_d, out_d):
    nc = tc.nc

    io = ctx.enter_context(tc.tile_pool(name="io", bufs=2))
    etvp = ctx.enter_context(tc.tile_pool(name="etv", bufs=2))
    xep = ctx.enter_context(tc.tile_pool(name="xe", bufs=2))
    outp = ctx.enter_context(tc.tile_pool(name="outp", bufs=2))
    consts = ctx.enter_context(tc.tile_pool(name="consts", bufs=1))
    small = ctx.enter_context(tc.tile_pool(name="small", bufs=2))
    # PSUM budget (8 banks): tv 3 bufs x 2 banks = 6, plus ONE bank
    # holding both samples' J/stat accumulators AND every phase-2
    # matmul output AND the warm-up target, packed as column ranges of
    # a single [P, 512] f32 tile.  The third tv buffer gives the
    # exp->xe->transpose round-trip enough slack that the Act exp
    # stream (the roofline) never stalls on it.
    tvps = ctx.enter_context(tc.tile_pool(name="tvps", bufs=3, space="PSUM"))
    jst = ctx.enter_context(tc.tile_pool(name="jst", bufs=1, space="PSUM"))

    tsb = {}   # (s, piece) -> text fp16 tile
    vsb = {}
    jtl = {}   # s -> [P, 132] f32 PSUM: J (0:128) + S_v/S_t/T (128:131)
    tvl = {}   # (s, g) -> transpose-group PSUM tile
    etl = {}   # (s, g) -> exp(group) SBUF tile
    xel = {}   # (s, g) -> t*e^t group tile

    # fp16 identity for the 128x128 PE transposes
    ident_h = consts.tile([P, P], FP16)

    def emit_loads(s, with_ident=False):
        # sample 0: text on Pool/SWDGE, vis on SP/HWDGE so both streams
        # interleave at the DMA device and group g's pair lands together.
        # sample 1: BOTH tensors on Pool/SWDGE - the SP queue issues
        # faster than SWDGE and its transfers would otherwise overtake
        # sample 0's text pieces in the DMA FIFO, starving the exp
        # stream (the critical engine) for several microseconds.
        for pi, (o, w) in enumerate(PIECES_S[s]):
            t = io.tile([P, w], FP16, tag=f"t{pi}", name=f"t{s}_{pi}")
            v = io.tile([P, w], FP16, tag=f"v{pi}", name=f"v{s}_{pi}")
            tsb[(s, pi)] = t
            vsb[(s, pi)] = v
            src_t = text_d[s, :, :, o:o + w].rearrange("c h n -> (c h) n")
            nc.gpsimd.dma_start(out=t, in_=src_t)
            src_v = vis_d[s, :, :, o:o + w].rearrange("c h n -> (c h) n")
            if s == 0:
                nc.sync.dma_start(out=v, in_=src_v)
            else:
                nc.gpsimd.dma_start(out=v, in_=src_v)
            if with_ident and pi == 0:
                # identity lands on the Pool queue right after the first
                # text piece so transposes aren't blocked behind the
                # whole load stream
                make_identity(nc, ident_h)

    def emit_transposes(s, g):
        off, w = GROUPS_S[s][g]
        pi, lo = GRP_SRC_S[s][g]
        t_src, v_src = tsb[(s, pi)], vsb[(s, pi)]
        nb = w // 128
        tv = tvps.tile([P, 2 * w], FP16, tag="tv", name=f"tv{s}_{g}")
        tvl[(s, g)] = tv
        for b in range(nb):
            c0 = lo + b * 128
            nc.tensor.transpose(tv[:, b * 128:(b + 1) * 128],
                                t_src[:, c0:c0 + 128], ident_h)
        for b in range(nb):
            c0 = lo + b * 128
            nc.tensor.transpose(tv[:, w + b * 128:w + (b + 1) * 128],
                                v_src[:, c0:c0 + 128], ident_h)

    def emit_exp_xe(s, g, split=False):
        off, w = GROUPS_S[s][g]
        tv = tvl[(s, g)]
        etv = etvp.tile([P, 2 * w], FP16, tag="etv", name=f"etv{s}_{g}")
        etl[(s, g)] = etv
        if split:
            # text half first so xe (and the J rhs) unblock earlier
            nc.scalar.activation(out=etv[:, 0:w], in_=tv[:, 0:w],
                                 func=AF.Exp)
            nc.scalar.activation(out=etv[:, w:2 * w], in_=tv[:, w:2 * w],
                                 func=AF.Exp)
        else:
            nc.scalar.activation(out=etv, in_=tv, func=AF.Exp)
        xe = xep.tile([P, w], FP16, tag="xe", name=f"xe{s}_{g}")
        xel[(s, g)] = xe
        # tensor_tensor gets the 2x DVE mode; this read also releases tv
        nc.vector.tensor_mul(xe, tv[:, 0:w], etv[:, 0:w])

    def _blk0(s, g):
        return GROUPS_S[s][g][0] // 128

    def emit_matmuls(s, g):
        off, w = GROUPS_S[s][g]
        etv, j_t = etl[(s, g)], jtl[s]
        for b in range(w // 128):
            k = _blk0(s, g) + b
            sp_f = (k == NBLK - 1)
            etT = etv[:, b * 128:(b + 1) * 128]
            evT = etv[:, w + b * 128:w + (b + 1) * 128]
            # all chains accumulate onto PSUM zeros with start=False
            nc.tensor.matmul(j_t[:, 0:128], lhsT=evT, rhs=etT,
                             start=False, stop=sp_f,
                             skip_group_check=True)
            nc.tensor.matmul(j_t[:, 128:129], lhsT=evT, rhs=ones128h,
                             start=False, stop=sp_f,
                             skip_group_check=True)
            nc.tensor.matmul(j_t[:, 129:130], lhsT=etT, rhs=ones128h,
                             start=False, stop=sp_f,
                             skip_group_check=True)

    def emit_tstats(s, g):
        # T = sum_n t*e^t column sums; emitted one group late so the PE
        # queue never blocks on the DVE xe product at the queue head
        off, w = GROUPS_S[s][g]
        xe, j_t = xel[(s, g)], jtl[s]
        for b in range(w // 128):
            k = _blk0(s, g) + b
            nc.tensor.matmul(j_t[:, 130:131],
                             lhsT=xe[:, b * 128:(b + 1) * 128],
                             rhs=ones128h, start=False,
                             stop=(k == NBLK - 1),
                             skip_group_check=True)

    def emit_phase2_stats(s):
        """negent/recips branch; independent of the J merge. Reads the
        Sv/St/T stat columns straight out of PSUM where possible."""
        parts = small.tile([P, 3], F32, tag="parts", name=f"parts{s}")
        nc.vector.tensor_copy(out=parts, in_=jtl[s][:, 128:131])
        sums_ps = jt_all[0:C, 456:459]
        nc.tensor.matmul(sums_ps, lhsT=hsum, rhs=parts, start=True,
                         stop=True)
        recips = small.tile([C, 3], F32, tag="recips", name=f"recips{s}")
        nc.vector.reciprocal(out=recips[:, 0:2], in_=sums_ps[:, 0:2])
        rst = recips[:, 1:2]
        nc.vector.tensor_scalar_mul(out=recips[:, 2:3], in0=recips[:, 0:1],
                                    scalar1=0.5)
        lnls = small.tile([C, 2], F32, tag="lnls", name=f"lnls{s}")
        nc.scalar.activation(out=lnls, in_=sums_ps[:, 0:2], func=AF.Ln)
        # fold the lt0 rescale constant ln(HW^2 / JSCALE) into lnSv so the
        # Ln of the raw joint runs on O(1) inputs (act table accuracy)
        nc.vector.tensor_scalar_add(out=lnls[:, 0:1], in0=lnls[:, 0:1],
                                    scalar1=-LNK0)
        # negent = T/St - lnSt; bias for the final Exp folds negent in:
        # biasv = -negent - (1 + HW*EPS)
        negent = small.tile([C, 1], F32, tag="negent", name=f"negent{s}")
        nc.vector.scalar_tensor_tensor(
            out=negent, in0=sums_ps[:, 2:3], scalar=rst, in1=lnls[:, 1:2],
            op0=ALU.mult, op1=ALU.subtract)
        biasv = small.tile([C, 1], F32, tag="biasv", name=f"biasv{s}")
        nc.vector.tensor_scalar(out=biasv, in0=negent, scalar1=-1.0,
                                scalar2=-(1.0 + HW * EPS), op0=ALU.mult,
                                op1=ALU.add)
        r2 = small.tile([C, 2], F32, tag="r2", name=f"r2{s}")
        nc.vector.memset(r2[:, 0:1], 1.0)
        nc.vector.tensor_copy(out=r2[:, 1:2], in_=lnls[:, 1:2])
        return recips, lnls, biasv, r2

    def emit_phase2_ji(s):
        """J-branch head: only needs the J matmul chain stop (emitted
        before the T-stat stop so it starts ~0.5us earlier).
        J64T[d,c] = sum_h JJ[(c,h),(d,h)]"""
        jj = small.tile([P, P], F32, tag="jj", name=f"jj{s}")
        nc.vector.tensor_copy(out=jj, in_=jtl[s][:, 0:128])
        jj_v = jj.rearrange("p (c h) -> p c h", h=2)
        j64t_ps = jt_all[0:C, 392:456]
        for h in range(2):
            nc.tensor.matmul(j64t_ps, lhsT=jj_v[:, :, h], rhs=esel[h],
                             start=(h == 0), stop=(h == 1))
        return j64t_ps

    def emit_phase2_lt0(s, j64t_ps):
        lt0 = small.tile([C, C], F32, tag="lt0", name=f"lt0{s}")
        nc.scalar.activation(out=lt0, in_=j64t_ps, func=AF.Ln,
                             scale=JSCALE)
        return lt0

    def emit_phase2_mi(s, stats, j64t_ps, lt0):
        """mi via the separable log:
        ln(HW^2*J/(Sv*St)) = ln(HW^2*Jraw) - lnSv_c - lnSt_d
        (the +EPS inside the reference log shifts values ~1e-9; dropped)
        """
        recips, lnls, biasv, r2 = stats
        rsv05 = recips[:, 2:3]
        rst = recips[:, 1:2]
        lnsv = lnls[:, 0:1]
        # PSUM->SBUF copy doubles as the rst_d row scaling
        j64t = small.tile([C, C], F32, tag="j64t_sb", name=f"j64t_sb{s}")
        nc.vector.tensor_scalar_mul(out=j64t, in0=j64t_ps, scalar1=rst)
        q = small.tile([C, C], F32, tag="q", name=f"q{s}")
        nc.vector.tensor_mul(q, lt0, j64t)
        ry_ps = jt_all[0:C, 459:461]
        nc.tensor.matmul(ry_ps, lhsT=j64t, rhs=r2, start=True, stop=True)
        u1_ps = jt_all[0:C, 461:462]
        nc.tensor.matmul(u1_ps, lhsT=q, rhs=ones64[:, 0:1], start=True,
                         stop=True)
        # z1 = ry0*lnsv + ry1 ; zz = (u1 - z1) * 0.5/Sv  (two-scalar ops)
        z1 = small.tile([C, 1], F32, tag="z1", name=f"z1{s}")
        nc.vector.tensor_scalar(out=z1, in0=ry_ps[:, 0:1], scalar1=lnsv,
                                scalar2=ry_ps[:, 1:2], op0=ALU.mult,
                                op1=ALU.add)
        zz = small.tile([C, 1], F32, tag="zz", name=f"zz{s}")
        nc.vector.tensor_scalar(out=zz, in0=u1_ps, scalar1=z1,
                                scalar2=rsv05, op0=ALU.subtract,
                                op1=ALU.mult)
        mib_ps = jt_all[0:C, 462:463]
        nc.tensor.matmul(mib_ps, lhsT=ones64, rhs=zz, start=True,
                         stop=True)
        return mib_ps

    def emit_phase2_g(s, mib_ps, stats):
        biasv = stats[2]
        g64 = small.tile([C, 1], F32, tag="g64", name=f"g64{s}")
        nc.scalar.activation(out=g64, in_=mib_ps, func=AF.Exp, scale=-1.0,
                             bias=biasv)
        nc.vector.tensor_scalar_add(out=g64, in0=g64, scalar1=1.0)
        nc.vector.reciprocal(out=g64, in_=g64)
        g_ps = jt_all[:, 463:464]
        nc.tensor.matmul(g_ps, lhsT=hsumT, rhs=g64, start=True, stop=True)
        g = small.tile([P, 1], F32, tag="g", name=f"g{s}")
        nc.vector.tensor_copy(out=g, in_=g_ps)
        return g

    otl = {}

    def emit_madd(s, g, ci, dg=None):
        pi, lo, hi, eng = MADD_S[s][ci]
        o, w = PIECES_S[s][pi]
        if (s, pi) not in otl:
            otl[(s, pi)] = outp.tile([P, w], FP16, tag=f"o{s}_{pi}",
                                     name=f"o{s}_{pi}")
        ot = otl[(s, pi)]
        if eng == "dve" or eng == "dvepool":
            # g*text at 4x (f32 ptr scalar exempt from the 2-byte rule),
            # then += vis at 2x (or on the idle Pool for dvepool)
            nc.vector.tensor_scalar_mul(out=ot[:, lo:hi],
                                        in0=tsb[(s, pi)][:, lo:hi],
                                        scalar1=g)
            add_eng = nc.vector if eng == "dve" else nc.gpsimd
            add_eng.tensor_add(ot[:, lo:hi], ot[:, lo:hi],
                               vsb[(s, pi)][:, lo:hi])
        else:
            # Dg@text + I@vis accumulated in PSUM, Act copies to fp16
            pm = tvps.tile([P, hi - lo], F32, tag="tv",
                           name=f"pm{s}_{ci}")
            nc.tensor.matmul(pm, lhsT=dg, rhs=tsb[(s, pi)][:, lo:hi],
                             start=True, stop=False)
            nc.tensor.matmul(pm, lhsT=ident_h, rhs=vsb[(s, pi)][:, lo:hi],
                             start=False, stop=True)
            nc.scalar.activation(out=ot[:, lo:hi], in_=pm, func=AF.Copy)

    store_ctr = [0]

    def emit_store(s, pi, lo=None, hi=None):
        # alternate SP/Pool queues so store issue never serializes on
        # one sequencer at the tail; lo/hi store a piece sub-range so
        # tail halves stream out as soon as their chunks finish
        o, w = PIECES_S[s][pi]
        if lo is None:
            lo, hi = 0, w
        dst = out_d[s, :, :, o + lo:o + hi].rearrange("c h n -> (c h) n")
        eng = nc.sync if store_ctr[0] % 2 == 0 else nc.gpsimd
        store_ctr[0] += 1
        eng.dma_start(out=dst, in_=otl[(s, pi)][:, lo:hi])

    # ---- emission ----
    # single-bank PSUM mega-tile: j0 0:132, j1 132:264, warm 264:392,
    # j64t 392:456, sums 456:459, ry 459:461, u1 461, mib 462, gbc 463
    jt_all = jst.tile([P, 512], F32, tag="jall", name="jall")
    for s in range(2):
        jtl[s] = jt_all[:, s * 132:(s + 1) * 132]

    # DVE-built constants + PE warm-up fodder (DVE is idle at t=0)
    ones128h = consts.tile([P, 1], FP16)
    nc.vector.memset(ones128h, 1.0)
    junk = consts.tile([P, 192], FP16)
    nc.vector.memset(junk, 0.0)

    emit_loads(0, with_ident=True)

    # nudge the PE p-state ramp before the first transposes (engine
    # init means PE can't start before ~2.4us; the first piece lands
    # ~2.9us, so just a few warms to leave the lowest p-state)
    warm_ps = jt_all[0:1, 264:392]
    for i in range(3):
        nc.tensor.matmul(warm_ps, lhsT=ones128h, rhs=junk[:, 0:128],
                         start=True, stop=True)

    # phase-2 constants on the Pool queue; they are only needed ~15us in
    hsum = consts.tile([P, C], F32)
    nc.gpsimd.memset(hsum, 0.0)
    for base in (0, -1):   # fill where p - 2c + base == 0
        nc.gpsimd.affine_select(out=hsum, in_=hsum,
                                compare_op=ALU.not_equal, fill=1.0,
                                base=base, pattern=[[-2, C]],
                                channel_multiplier=1)
    esel = []
    for h in range(2):     # E_h[p, c] = 1 iff p == 2c + h
        e = consts.tile([P, C], F32, tag=f"esel{h}", name=f"esel{h}")
        nc.gpsimd.memset(e, 0.0)
        nc.gpsimd.affine_select(out=e, in_=e, compare_op=ALU.not_equal,
                                fill=1.0, base=-h, pattern=[[-2, C]],
                                channel_multiplier=1)
        esel.append(e)
    hsumT = consts.tile([C, P], F32)
    nc.gpsimd.memset(hsumT, 0.0)
    for base in (0, -1):   # fill where p - 2c + base == 0
        nc.gpsimd.affine_select(out=hsumT, in_=hsumT,
                                compare_op=ALU.not_equal, fill=1.0,
                                base=base, pattern=[[1, P]],
                                channel_multiplier=-2)
    ones64 = consts.tile([C, C], F32)
    nc.gpsimd.memset(ones64, 1.0)

    NG0 = len(GROUPS_S[0])
    NG1 = len(GROUPS_S[1])

    # sample 0, software-pipelined: transposes TWO groups ahead of the
    # exp (3 tv buffers), so group g's J matmuls waiting on exp(g) at
    # the PE queue head never block the transposes of group g+2;
    # T-stat matmuls one group behind (they wait on DVE xe)
    emit_transposes(0, 0)
    emit_transposes(0, 1)
    emit_transposes(0, 2)
    emit_exp_xe(0, 0, split=True)
    emit_matmuls(0, 0)
    for g in range(1, NG0 - 2):
        emit_transposes(0, g + 2)
        emit_exp_xe(0, g)
        emit_matmuls(0, g)
        emit_tstats(0, g - 1)
    emit_exp_xe(0, NG0 - 2)
    emit_matmuls(0, NG0 - 2)
    emit_tstats(0, NG0 - 3)
    emit_exp_xe(0, NG0 - 1)
    emit_matmuls(0, NG0 - 1)
    ji0 = emit_phase2_ji(0)       # J branch head right at J-stop
    emit_tstats(0, NG0 - 2)
    emit_tstats(0, NG0 - 1)

    emit_loads(1)

    # sample 1 pipelined; sample-0 phase 2 is emitted AFTER exp(1,0) so
    # its (dependency-blocked) Act ops never head-of-line block the exp
    # stream; its Act ops then slot between sample-1 exps
    emit_transposes(1, 0)
    emit_transposes(1, 1)
    emit_transposes(1, 2)
    emit_exp_xe(1, 0)
    emit_matmuls(1, 0)

    stats0 = emit_phase2_stats(0)
    lt00 = emit_phase2_lt0(0, ji0)
    mib0 = emit_phase2_mi(0, stats0, ji0, lt00)

    emit_transposes(1, 3)
    emit_exp_xe(1, 1)
    emit_matmuls(1, 1)
    emit_tstats(1, 0)

    g0 = emit_phase2_g(0, mib0, stats0)

    # s0 madd chunks drip into the group iterations AFTER the xe
    # product; Pool-assisted chunks go first (their Pool adds must
    # clear the Pool queue before phase-2(1) needs it), and the short
    # 512-group iterations at the tail carry at most one chunk
    madd0 = list(range(len(MADD_S[0])))
    last_chunk = {}
    for ci, (pi, lo, hi, eng) in enumerate(MADD_S[0]):
        last_chunk[pi] = ci
    DRIP_PLAN = [[8, 9, 10], [11, 12, 13], [14, 15, 0], [1, 2, 3], [4, 5]]

    def drip(cis):
        for ci in cis:
            emit_madd(0, g0, ci)
            pi = MADD_S[0][ci][0]
            if last_chunk[pi] == ci:
                emit_store(0, pi)

    for g in range(2, NG1 - 2):
        emit_transposes(1, g + 2)
        emit_exp_xe(1, g)
        emit_matmuls(1, g)
        emit_tstats(1, g - 1)
        drip(DRIP_PLAN[g - 2])
    emit_exp_xe(1, NG1 - 2)
    emit_matmuls(1, NG1 - 2)
    emit_tstats(1, NG1 - 3)
    drip([6])
    emit_exp_xe(1, NG1 - 1)
    emit_matmuls(1, NG1 - 1)
    ji1 = emit_phase2_ji(1)       # J branch head right at J-stop
    emit_tstats(1, NG1 - 2)
    emit_tstats(1, NG1 - 1)

    stats1 = emit_phase2_stats(1)
    lt01 = emit_phase2_lt0(1, ji1)
    drip([7])
    mib1 = emit_phase2_mi(1, stats1, ji1, lt01)
    g1 = emit_phase2_g(1, mib1, stats1)

    # tail: PE+Act chunks and DVE chunks run in parallel; each 1024-col
    # half is stored the moment its chunks are written.  Dg = diag(g1)
    # feeds the PE path (4x tensor_scalar off ident)
    dg1 = small.tile([P, P], FP16, tag="dg", name="dg1")
    nc.vector.tensor_scalar_mul(out=dg1, in0=ident_h, scalar1=g1)
    for ci in (5, 6):                 # pe: p2[1024:2048]
        emit_madd(1, g1, ci, dg=dg1)
    emit_madd(1, g1, 4)               # dve: p2[0:1024]
    emit_store(1, 2, 0, 1024)
    emit_store(1, 2, 1024, 2048)
    for ci in (7, 8):                 # pe: p3[0:1024]
        emit_madd(1, g1, ci, dg=dg1)
    emit_madd(1, g1, 0)               # dve: p0[0:1024]
    emit_store(1, 3, 0, 1024)
    emit_store(1, 0, 0, 1024)
    for ci in (9, 10):                # pe: p3[1024:2048]
        emit_madd(1, g1, ci, dg=dg1)
    emit_madd(1, g1, 1)               # dve: p0[1024:2048]
    emit_store(1, 3, 1024, 2048)
    emit_store(1, 0, 1024, 2048)
    emit_madd(1, g1, 2)               # dve: p1[0:1024]
    emit_store(1, 1, 0, 1024)
    emit_madd(1, g1, 3)               # dve: p1[1024:2048]
    emit_store(1, 1, 1024, 2048)


_PROGRAM = None


def _get_program():
    global _PROGRAM
    if _PROGRAM is None:
        _PROGRAM = _build_program()
    return _PROGRAM


def kernel(vis_feat: np.ndarray, text_feat: np.ndarray) -> np.ndarray:
    nc = _get_program()
    vis = np.ascontiguousarray(vis_feat, dtype=np.float16)
    text = np.ascontiguousarray(text_feat, dtype=np.float16)
    bpc = B // NCORES
    in_maps = [
        {
            "vis": vis[i * bpc:(i + 1) * bpc].reshape(bpc, C, 2, HH),
            "text": text[i * bpc:(i + 1) * bpc].reshape(bpc, C, 2, HH),
        }
        for i in range(NCORES)
    ]
    res = run_bass_kernel_spmd(nc, in_maps, list(range(NCORES)))
    out = np.concatenate(
        [np.asarray(r["out"]).reshape(bpc, C, H, W) for r in res.results],
        axis=0)
    return out.astype(np.float32)


# revision 34
# speedup vs baseline: 1.0882x; 1.0399x over previous
"""EntropyGuidance Trainium2 kernel, fp16-I/O Act-roofline variant.

Each core handles 2 samples (B=16 over 8 cores), each sample packed as
[128 partitions = 64 channels x 2 HW-halves (p = 2c+h), 8192 free].

The Activation engine is the roofline here (~31us of exp work that no
other engine can run), so the schedule keeps Act 100% fed:
  - fp16 on both DMA directions (host casts): loads ride Pool/SWDGE
    (text) + SP/HWDGE (vis), stores SP/HWDGE; DMA busy ~35us.
  - PE is warmed with ~8 wide dummy matmuls before the first transpose
    so the p-state ramp never doubles transpose time mid-pipeline.
  - sample 0 leads with two 512-col groups so the first exp starts
    ~1us earlier; the exp->xe->transpose PSUM round-trip is kept under
    2x the exp stage time (2 tv buffers).
  - phase-2 Act ops are emitted AFTER the next sample's first exp so a
    blocked Ln never head-of-line blocks the exp stream.
  - out = vis + g*text is split DVE (tensor_scalar_mul 4x + tensor_add
    2x, 0.78 ns/col) / Pool (one-op scalar_tensor_tensor, 1.39 ns/col)
    so the tail after g1 shrinks and DVE never starves the xe products.
"""

import sys

sys.path.insert(0, "/opt/trn_rl_repo")

import math
from contextlib import ExitStack

import numpy as np

import concourse.bacc as bacc
import concourse.tile as tile
from concourse import mybir
from concourse.bass_utils import run_bass_kernel_spmd
from concourse.masks import make_identity

if not hasattr(bacc, "_orig_get_act_tables"):
    bacc._orig_get_act_tables = bacc.get_activation_tables


def _lnexp_only_tables(module_arch):
    tabs = bacc._orig_get_act_tables(module_arch)
    return {
        name: (funcs if name == "natural_log_exp_and_others" else set())
        for name, funcs in tabs.items()
    }


bacc.get_activation_tables = _lnexp_only_tables

F32 = mybir.dt.float32
FP16 = mybir.dt.float16
AF = mybir.ActivationFunctionType
ALU = mybir.AluOpType

B, C, H, W = 16, 64, 128, 128
HW = H * W                      # 16384
HH = HW // 2                    # 8192 per half
NCORES = 8
P = 128                         # partitions = 64 channels x 2 halves
EPS = 1e-9

# per-sample load pieces along the 8192 free axis
PIECES_S = [
    [(0, 512), (512, 512), (1024, 1024), (2048, 2048), (4096, 2048),
     (6144, 2048)],
    [(0, 2048), (2048, 2048), (4096, 2048), (6144, 2048)],
]
# transpose/exp groups (off, width); J blocks per group = width/128.
# sample 1 ends with two 512 groups so the last exp -> J/T-stop chain
# (which gates g1 and the tail stores) is as short as possible
GROUPS_S = [
    [(0, 512), (512, 512)] + [(1024 + g * 1024, 1024) for g in range(7)],
    [(g * 1024, 1024) for g in range(7)] + [(7168, 512), (7680, 512)],
]
# madd work split: (piece, col_lo, col_hi, engine)
# DVE gets 2-op (0.78/col) chunks; sample-0 chunks are 512 wide so an
# in-flight chunk never delays an xe product by more than ~0.4us.
# "pe" chunks run Dg@text + I@vis on the Tensor engine into PSUM with
# an Act PSUM->SBUF fp16 copy (both engines idle at the tail; PE
# chunks are 512 wide since a matmul output must fit one PSUM bank)
MADD_S = [
    # sample 0: 16 x 512-col chunks; "dvepool" = DVE 4x tensor_scalar
    # for g*text then the +vis add on the idle Pool engine, so each
    # group iteration absorbs ~3 chunks without out-pacing the 1.9us
    # Act stage on DVE
    [(0, 0, 512, "dve"), (1, 0, 512, "dve"),
     (2, 0, 512, "dve"), (2, 512, 1024, "dve"),
     (3, 0, 512, "dve"), (3, 512, 1024, "dve"),
     (3, 1024, 1536, "dvepool"), (3, 1536, 2048, "dvepool"),
     (4, 0, 512, "dvepool"), (4, 512, 1024, "dvepool"),
     (4, 1024, 1536, "dvepool"), (4, 1536, 2048, "dvepool"),
     (5, 0, 512, "dvepool"), (5, 512, 1024, "dvepool"),
     (5, 1024, 1536, "dvepool"), (5, 1536, 2048, "dvepool")],
    # sample 1 (tail): DVE 5120 cols, PE+Act 3072 cols in parallel
    [(0, 0, 1024, "dve"), (0, 1024, 2048, "dve"),
     (1, 0, 1024, "dve"), (1, 1024, 2048, "dve"),
     (2, 0, 1024, "dve"),
     (2, 1024, 1536, "pe"), (2, 1536, 2048, "pe"),
     (3, 0, 512, "pe"), (3, 512, 1024, "pe"),
     (3, 1024, 1536, "pe"), (3, 1536, 2048, "pe")],
]


def _grp_src(pieces, groups):
    out = []
    for off, w in groups:
        for pi, (o, pw) in enumerate(pieces):
            if o <= off and off + w <= o + pw:
                out.append((pi, off - o))
                break
        else:
            raise AssertionError((off, w))
    return out


GRP_SRC_S = [_grp_src(PIECES_S[s], GROUPS_S[s]) for s in range(2)]
NBLK = 64                       # J blocks per sample
# Ln(joint) rescale: raw J entries are ~Sv*St/HW ~ 4.5e4, so scale the act
# Ln input to ~1.4 and add ln(HW^2 / JSCALE) back via lnSv
JSCALE = 1.0 / 32768.0
LNK0 = math.log(float(HW) * float(HW) / JSCALE)


def _build_program():
    nc = bacc.Bacc()
    vis_d = nc.declare_dram_parameter("vis", [2, 2, C, HH], FP16,
                                      isOutput=False)
    text_d = nc.declare_dram_parameter("text", [2, 2, C, HH], FP16,
                                       isOutput=False)
    out_d = nc.declare_dram_parameter("out", [2, 2, C, HH], FP16,
                                      isOutput=True)

    with ExitStack() as ctx:
        tc = ctx.enter_context(tile.TileContext(nc))
        _emit(ctx, tc, vis_d, text_d, out_d)
    nc.finalize()
    return nc


def _emit(ctx: ExitStack, tc: tile.TileContext, vis_d, text_d, out_d):
    nc = tc.nc

    io = ctx.enter_context(tc.tile_pool(name="io", bufs=2))
    etvp = ctx.enter_context(tc.tile_pool(name="etv", bufs=2))
    xep = ctx.enter_context(tc.tile_pool(name="xe", bufs=2))
    outp = ctx.enter_context(tc.tile_pool(name="outp", bufs=2))
    consts = ctx.enter_context(tc.tile_pool(name="consts", bufs=1))
    small = ctx.enter_context(tc.tile_pool(name="small", bufs=2))
    # PSUM budget (8 banks): tv 3 bufs x 2 banks = 6, plus ONE bank
    # holding both samples' J/stat accumulators AND every phase-2
    # matmul output AND the warm-up target, packed as column ranges of
    # a single [P, 512] f32 tile.  The third tv buffer gives the
    # exp->xe->transpose round-trip enough slack that the Act exp
    # stream (the roofline) never stalls on it.
    tvps = ctx.enter_context(tc.tile_pool(name="tvps", bufs=3, space="PSUM"))
    jst = ctx.enter_context(tc.tile_pool(name="jst", bufs=1, space="PSUM"))

    tsb = {}   # (s, piece) -> text fp16 tile
    vsb = {}
    jtl = {}   # s -> [P, 132] f32 PSUM: J (0:128) + S_v/S_t/T (128:131)
    tvl = {}   # (s, g) -> transpose-group PSUM tile
    etl = {}   # (s, g) -> exp(group) SBUF tile
    xel = {}   # (s, g) -> t*e^t group tile

    # fp16 identity for the 128x128 PE transposes
    ident_h = consts.tile([P, P], FP16)

    def emit_loads(s, with_ident=False):
        # sample 0: text on Pool/SWDGE, vis on SP/HWDGE so both streams
        # interleave at the DMA device and group g's pair lands together.
        # sample 1: BOTH tensors on Pool/SWDGE - the SP queue issues
        # faster than SWDGE and its transfers would otherwise overtake
        # sample 0's text pieces in the DMA FIFO, starving the exp
        # stream (the critical engine) for several microseconds.
        for pi, (o, w) in enumerate(PIECES_S[s]):
            t = io.tile([P, w], FP16, tag=f"t{pi}", name=f"t{s}_{pi}")
            v = io.tile([P, w], FP16, tag=f"v{pi}", name=f"v{s}_{pi}")
            tsb[(s, pi)] = t
            vsb[(s, pi)] = v
            src_t = text_d[s, :, :, o:o + w].rearrange("h c n -> (h c) n")
            nc.gpsimd.dma_start(out=t, in_=src_t)
            src_v = vis_d[s, :, :, o:o + w].rearrange("h c n -> (h c) n")
            if s == 0:
                nc.sync.dma_start(out=v, in_=src_v)
            else:
                nc.gpsimd.dma_start(out=v, in_=src_v)
            if with_ident and pi == 0:
                # identity lands on the Pool queue right after the first
                # text piece so transposes aren't blocked behind the
                # whole load stream
                make_identity(nc, ident_h)

    def emit_transposes(s, g):
        off, w = GROUPS_S[s][g]
        pi, lo = GRP_SRC_S[s][g]
        t_src, v_src = tsb[(s, pi)], vsb[(s, pi)]
        nb = w // 128
        tv = tvps.tile([P, 2 * w], FP16, tag="tv", name=f"tv{s}_{g}")
        tvl[(s, g)] = tv
        for b in range(nb):
            c0 = lo + b * 128
            nc.tensor.transpose(tv[:, b * 128:(b + 1) * 128],
                                t_src[:, c0:c0 + 128], ident_h)
        for b in range(nb):
            c0 = lo + b * 128
            nc.tensor.transpose(tv[:, w + b * 128:w + (b + 1) * 128],
                                v_src[:, c0:c0 + 128], ident_h)

    def emit_exp_xe(s, g, split=False):
        off, w = GROUPS_S[s][g]
        tv = tvl[(s, g)]
        etv = etvp.tile([P, 2 * w], FP16, tag="etv", name=f"etv{s}_{g}")
        etl[(s, g)] = etv
        if split:
            # text half first so xe (and the J rhs) unblock earlier
            nc.scalar.activation(out=etv[:, 0:w], in_=tv[:, 0:w],
                                 func=AF.Exp)
            nc.scalar.activation(out=etv[:, w:2 * w], in_=tv[:, w:2 * w],
                                 func=AF.Exp)
        else:
            nc.scalar.activation(out=etv, in_=tv, func=AF.Exp)
        xe = xep.tile([P, w], FP16, tag="xe", name=f"xe{s}_{g}")
        xel[(s, g)] = xe
        # tensor_tensor gets the 2x DVE mode; this read also releases tv
        nc.vector.tensor_mul(xe, tv[:, 0:w], etv[:, 0:w])

    def _blk0(s, g):
        return GROUPS_S[s][g][0] // 128

    def emit_matmuls(s, g):
        off, w = GROUPS_S[s][g]
        etv, j_t = etl[(s, g)], jtl[s]
        for b in range(w // 128):
            k = _blk0(s, g) + b
            sp_f = (k == NBLK - 1)
            etT = etv[:, b * 128:(b + 1) * 128]
            evT = etv[:, w + b * 128:w + (b + 1) * 128]
            # all chains accumulate onto PSUM zeros with start=False;
            # lhsT=etT so JJ[pt, pv] rows are text-side (the d index
            # the phase-2 rst row-scale and d-contraction need)
            nc.tensor.matmul(j_t[:, 0:128], lhsT=etT, rhs=evT,
                             start=False, stop=sp_f,
                             skip_group_check=True)
            nc.tensor.matmul(j_t[:, 128:129], lhsT=evT, rhs=ones128h,
                             start=False, stop=sp_f,
                             skip_group_check=True)
            nc.tensor.matmul(j_t[:, 129:130], lhsT=etT, rhs=ones128h,
                             start=False, stop=sp_f,
                             skip_group_check=True)

    def emit_tstats(s, g):
        # T = sum_n t*e^t column sums; emitted one group late so the PE
        # queue never blocks on the DVE xe product at the queue head
        off, w = GROUPS_S[s][g]
        xe, j_t = xel[(s, g)], jtl[s]
        for b in range(w // 128):
            k = _blk0(s, g) + b
            nc.tensor.matmul(j_t[:, 130:131],
                             lhsT=xe[:, b * 128:(b + 1) * 128],
                             rhs=ones128h, start=False,
                             stop=(k == NBLK - 1),
                             skip_group_check=True)

    def emit_phase2_stats(s):
        """negent/recips branch; independent of the J merge. Reads the
        Sv/St/T stat columns straight out of PSUM where possible."""
        # h-fold via a partition-strided DVE add straight out of PSUM:
        # sums[c] = stats[2c] + stats[2c+1], one op, no PE round-trip
        sh = small.tile([C, 3], F32, tag="sumh", name=f"sumh{s}")
        nc.vector.tensor_copy(out=sh, in_=jtl[s][C:P, 128:131])
        sums = small.tile([C, 3], F32, tag="sums", name=f"sums{s}")
        nc.vector.tensor_add(sums, sh, jtl[s][0:C, 128:131])
        recips = small.tile([C, 3], F32, tag="recips", name=f"recips{s}")
        nc.vector.reciprocal(out=recips[:, 0:2], in_=sums[:, 0:2])
        rst = recips[:, 1:2]
        nc.vector.tensor_scalar_mul(out=recips[:, 2:3], in0=recips[:, 0:1],
                                    scalar1=0.5)
        lnls = small.tile([C, 2], F32, tag="lnls", name=f"lnls{s}")
        nc.scalar.activation(out=lnls, in_=sums[:, 0:2], func=AF.Ln)
        # fold the lt0 rescale constant ln(HW^2 / JSCALE) into lnSv so the
        # Ln of the raw joint runs on O(1) inputs (act table accuracy)
        nc.vector.tensor_scalar_add(out=lnls[:, 0:1], in0=lnls[:, 0:1],
                                    scalar1=-LNK0)
        # negent = T/St - lnSt; bias for the final Exp folds negent in:
        # biasv = -negent - (1 + HW*EPS)
        negent = small.tile([C, 1], F32, tag="negent", name=f"negent{s}")
        nc.vector.scalar_tensor_tensor(
            out=negent, in0=sums[:, 2:3], scalar=rst, in1=lnls[:, 1:2],
            op0=ALU.mult, op1=ALU.subtract)
        biasv = small.tile([C, 1], F32, tag="biasv", name=f"biasv{s}")
        nc.vector.tensor_scalar(out=biasv, in0=negent, scalar1=-1.0,
                                scalar2=-(1.0 + HW * EPS), op0=ALU.mult,
                                op1=ALU.add)
        r2 = small.tile([C, 2], F32, tag="r2", name=f"r2{s}")
        nc.vector.memset(r2[:, 0:1], 1.0)
        nc.vector.tensor_copy(out=r2[:, 1:2], in_=lnls[:, 1:2])
        return recips, lnls, biasv, r2

    def emit_phase2_ji(s):
        """J-branch head: only needs the J matmul chain stop (emitted
        before the T-stat stop so it starts ~0.5us earlier).
        J64T[d,c] = sum_h JJ[(c,h),(d,h)]"""
        jh = small.tile([C, C], F32, tag="jjh", name=f"jjh{s}")
        nc.vector.tensor_copy(out=jh, in_=jtl[s][C:P, C:P])
        j64sum = small.tile([C, C], F32, tag="jj", name=f"jj{s}")
        nc.vector.tensor_add(j64sum, jh, jtl[s][0:C, 0:C])
        return j64sum

    def emit_phase2_lt0(s, j64sum):
        lt0 = small.tile([C, C], F32, tag="lt0", name=f"lt0{s}")
        nc.scalar.activation(out=lt0, in_=j64sum, func=AF.Ln,
                             scale=JSCALE)
        return lt0

    def emit_phase2_mi(s, stats, j64sum, lt0):
        """mi via the separable log:
        ln(HW^2*J/(Sv*St)) = ln(HW^2*Jraw) - lnSv_c - lnSt_d
        (the +EPS inside the reference log shifts values ~1e-9; dropped)
        """
        recips, lnls, biasv, r2 = stats
        rsv05 = recips[:, 2:3]
        rst = recips[:, 1:2]
        lnsv = lnls[:, 0:1]
        # PSUM->SBUF copy doubles as the rst_d row scaling
        j64t = small.tile([C, C], F32, tag="j64t_sb", name=f"j64t_sb{s}")
        nc.vector.tensor_scalar_mul(out=j64t, in0=j64sum, scalar1=rst)
        q = small.tile([C, C], F32, tag="q", name=f"q{s}")
        nc.vector.tensor_mul(q, lt0, j64t)
        ry_ps = jt_all[0:C, 459:461]
        nc.tensor.matmul(ry_ps, lhsT=j64t, rhs=r2, start=True, stop=True)
        u1_ps = jt_all[0:C, 461:462]
        nc.tensor.matmul(u1_ps, lhsT=q, rhs=ones64[:, 0:1], start=True,
                         stop=True)
        # z1 = ry0*lnsv + ry1 ; zz = (u1 - z1) * 0.5/Sv  (two-scalar ops)
        z1 = small.tile([C, 1], F32, tag="z1", name=f"z1{s}")
        nc.vector.tensor_scalar(out=z1, in0=ry_ps[:, 0:1], scalar1=lnsv,
                                scalar2=ry_ps[:, 1:2], op0=ALU.mult,
                                op1=ALU.add)
        zz = small.tile([C, 1], F32, tag="zz", name=f"zz{s}")
        nc.vector.tensor_scalar(out=zz, in0=u1_ps, scalar1=z1,
                                scalar2=rsv05, op0=ALU.subtract,
                                op1=ALU.mult)
        mib_ps = jt_all[0:C, 462:463]
        nc.tensor.matmul(mib_ps, lhsT=ones64, rhs=zz, start=True,
                         stop=True)
        return mib_ps

    def emit_phase2_g(s, mib_ps, stats):
        biasv = stats[2]
        g64 = small.tile([C, 1], F32, tag="g64", name=f"g64{s}")
        nc.scalar.activation(out=g64, in_=mib_ps, func=AF.Exp, scale=-1.0,
                             bias=biasv)
        nc.vector.tensor_scalar_add(out=g64, in0=g64, scalar1=1.0)
        nc.vector.reciprocal(out=g64, in_=g64)
        g = small.tile([P, 1], F32, tag="g", name=f"g{s}")
        nc.vector.tensor_copy(out=g[0:C, :], in_=g64)
        nc.vector.tensor_copy(out=g[C:P, :], in_=g64)
        return g, g64

    otl = {}

    def emit_madd(s, g, ci, dg=None):
        pi, lo, hi, eng = MADD_S[s][ci]
        o, w = PIECES_S[s][pi]
        if (s, pi) not in otl:
            otl[(s, pi)] = outp.tile([P, w], FP16, tag=f"o{s}_{pi}",
                                     name=f"o{s}_{pi}")
        ot = otl[(s, pi)]
        if eng == "dve" or eng == "dvepool":
            # g*text at 4x (f32 ptr scalar exempt from the 2-byte rule),
            # then += vis at 2x (or on the idle Pool for dvepool)
            nc.vector.tensor_scalar_mul(out=ot[:, lo:hi],
                                        in0=tsb[(s, pi)][:, lo:hi],
                                        scalar1=g)
            add_eng = nc.vector if eng == "dve" else nc.gpsimd
            add_eng.tensor_add(ot[:, lo:hi], ot[:, lo:hi],
                               vsb[(s, pi)][:, lo:hi])
        else:
            # Dg@text + I@vis accumulated in PSUM, Act copies to fp16
            pm = tvps.tile([P, hi - lo], F32, tag="tv",
                           name=f"pm{s}_{ci}")
            nc.tensor.matmul(pm, lhsT=dg, rhs=tsb[(s, pi)][:, lo:hi],
                             start=True, stop=False)
            nc.tensor.matmul(pm, lhsT=ident_h, rhs=vsb[(s, pi)][:, lo:hi],
                             start=False, stop=True)
            nc.scalar.activation(out=ot[:, lo:hi], in_=pm, func=AF.Copy)

    store_ctr = [0]

    def emit_store(s, pi, lo=None, hi=None):
        # alternate SP/Pool queues so store issue never serializes on
        # one sequencer at the tail; lo/hi store a piece sub-range so
        # tail halves stream out as soon as their chunks finish
        o, w = PIECES_S[s][pi]
        if lo is None:
            lo, hi = 0, w
        dst = out_d[s, :, :, o + lo:o + hi].rearrange("h c n -> (h c) n")
        eng = nc.sync if store_ctr[0] % 2 == 0 else nc.gpsimd
        store_ctr[0] += 1
        eng.dma_start(out=dst, in_=otl[(s, pi)][:, lo:hi])

    # ---- emission ----
    # single-bank PSUM mega-tile: j0 0:132, j1 132:264, warm 264:392,
    # j64t 392:456, sums 456:459, ry 459:461, u1 461, mib 462, gbc 463
    jt_all = jst.tile([P, 512], F32, tag="jall", name="jall")
    for s in range(2):
        jtl[s] = jt_all[:, s * 132:(s + 1) * 132]

    # DVE-built constants + PE warm-up fodder (DVE is idle at t=0)
    ones128h = consts.tile([P, 1], FP16)
    nc.vector.memset(ones128h, 1.0)
    junk = consts.tile([P, 192], FP16)
    nc.vector.memset(junk, 0.0)

    emit_loads(0, with_ident=True)

    # nudge the PE p-state ramp before the first transposes (engine
    # init means PE can't start before ~2.4us; the first piece lands
    # ~2.9us, so just a few warms to leave the lowest p-state)
    warm_ps = jt_all[0:1, 264:392]
    for i in range(3):
        nc.tensor.matmul(warm_ps, lhsT=ones128h, rhs=junk[:, 0:128],
                         start=True, stop=True)

    ones64 = consts.tile([C, C], F32)
    nc.gpsimd.memset(ones64, 1.0)

    NG0 = len(GROUPS_S[0])
    NG1 = len(GROUPS_S[1])

    # sample 0, software-pipelined: transposes TWO groups ahead of the
    # exp (3 tv buffers), so group g's J matmuls waiting on exp(g) at
    # the PE queue head never block the transposes of group g+2;
    # T-stat matmuls one group behind (they wait on DVE xe)
    emit_transposes(0, 0)
    emit_transposes(0, 1)
    emit_transposes(0, 2)
    emit_exp_xe(0, 0, split=True)
    emit_matmuls(0, 0)
    for g in range(1, NG0 - 2):
        emit_transposes(0, g + 2)
        emit_exp_xe(0, g)
        emit_matmuls(0, g)
        emit_tstats(0, g - 1)
    emit_exp_xe(0, NG0 - 2)
    emit_matmuls(0, NG0 - 2)
    emit_tstats(0, NG0 - 3)
    emit_exp_xe(0, NG0 - 1)
    emit_matmuls(0, NG0 - 1)
    ji0 = emit_phase2_ji(0)       # J branch head right at J-stop
    emit_tstats(0, NG0 - 2)
    emit_tstats(0, NG0 - 1)

    emit_loads(1)

    # sample 1 pipelined; sample-0 phase 2 is emitted AFTER exp(1,0) so
    # its (dependency-blocked) Act ops never head-of-line block the exp
    # stream; its Act ops then slot between sample-1 exps
    emit_transposes(1, 0)
    emit_transposes(1, 1)
    emit_transposes(1, 2)
    emit_exp_xe(1, 0)
    emit_matmuls(1, 0)

    stats0 = emit_phase2_stats(0)
    lt00 = emit_phase2_lt0(0, ji0)
    mib0 = emit_phase2_mi(0, stats0, ji0, lt00)

    emit_transposes(1, 3)
    emit_exp_xe(1, 1)
    emit_matmuls(1, 1)
    emit_tstats(1, 0)

    g0, g64_0 = emit_phase2_g(0, mib0, stats0)

    # s0 madd chunks drip into the group iterations AFTER the xe
    # product; Pool-assisted chunks go first (their Pool adds must
    # clear the Pool queue before phase-2(1) needs it), and the short
    # 512-group iterations at the tail carry at most one chunk
    remaining = {}
    for ci, (pi, lo, hi, eng) in enumerate(MADD_S[0]):
        remaining[pi] = remaining.get(pi, 0) + 1
    DRIP_PLAN = [[8, 9, 10, 14], [11, 12, 13, 15], [6, 7, 0], [1, 2, 3], [4, 5]]

    def drip(cis):
        for ci in cis:
            emit_madd(0, g0, ci)
            pi = MADD_S[0][ci][0]
            remaining[pi] -= 1
            if remaining[pi] == 0:
                emit_store(0, pi)

    for g in range(2, NG1 - 2):
        emit_transposes(1, g + 2)
        emit_exp_xe(1, g)
        emit_matmuls(1, g)
        emit_tstats(1, g - 1)
        drip(DRIP_PLAN[g - 2])
    emit_exp_xe(1, NG1 - 2)
    emit_matmuls(1, NG1 - 2)
    emit_tstats(1, NG1 - 3)
    emit_exp_xe(1, NG1 - 1)
    emit_matmuls(1, NG1 - 1)
    ji1 = emit_phase2_ji(1)       # J branch head right at J-stop
    emit_tstats(1, NG1 - 2)
    emit_tstats(1, NG1 - 1)

    stats1 = emit_phase2_stats(1)
    lt01 = emit_phase2_lt0(1, ji1)
    mib1 = emit_phase2_mi(1, stats1, ji1, lt01)
    g1, g64_1 = emit_phase2_g(1, mib1, stats1)

    # tail: PE+Act chunks and DVE chunks run in parallel; each 1024-col
    # half is stored the moment its chunks are written.  Dg = diag(g1)
    # feeds the PE path (4x tensor_scalar off ident)
    dg1 = small.tile([P, P], FP16, tag="dg", name="dg1")
    nc.vector.tensor_scalar_mul(out=dg1, in0=ident_h, scalar1=g1)
    for ci in (5, 6):                 # pe: p2[1024:2048]
        emit_madd(1, g1, ci, dg=dg1)
    emit_madd(1, g1, 4)               # dve: p2[0:1024]
    emit_store(1, 2, 0, 1024)
    emit_store(1, 2, 1024, 2048)
    for ci in (7, 8):                 # pe: p3[0:1024]
        emit_madd(1, g1, ci, dg=dg1)
    emit_madd(1, g1, 0)               # dve: p0[0:1024]
    emit_store(1, 3, 0, 1024)
    emit_store(1, 0, 0, 1024)
    for ci in (9, 10):                # pe: p3[1024:2048]
        emit_madd(1, g1, ci, dg=dg1)
    emit_madd(1, g1, 1)               # dve: p0[1024:2048]
    emit_store(1, 3, 1024, 2048)
    emit_store(1, 0, 1024, 2048)
    emit_madd(1, g1, 2)               # dve: p1[0:1024]
    emit_store(1, 1, 0, 1024)
    emit_madd(1, g1, 3)               # dve: p1[1024:2048]
    emit_store(1, 1, 1024, 2048)


_PROGRAM = None


def _get_program():
    global _PROGRAM
    if _PROGRAM is None:
        _PROGRAM = _build_program()
    return _PROGRAM


def kernel(vis_feat: np.ndarray, text_feat: np.ndarray) -> np.ndarray:
    nc = _get_program()
    vis = np.ascontiguousarray(vis_feat, dtype=np.float16)
    text = np.ascontiguousarray(text_feat, dtype=np.float16)
    bpc = B // NCORES
    in_maps = [
        {
            "vis": np.ascontiguousarray(
                vis[i * bpc:(i + 1) * bpc].reshape(bpc, C, 2, HH)
                .transpose(0, 2, 1, 3)),
            "text": np.ascontiguousarray(
                text[i * bpc:(i + 1) * bpc].reshape(bpc, C, 2, HH)
                .transpose(0, 2, 1, 3)),
        }
        for i in range(NCORES)
    ]
    res = run_bass_kernel_spmd(nc, in_maps, list(range(NCORES)))
    out = np.concatenate(
        [np.asarray(r["out"]).reshape(bpc, 2, C, HH).transpose(0, 2, 1, 3)
         .reshape(bpc, C, H, W) for r in res.results],
        axis=0)
    return out.astype(np.float32)


# revision 36
# speedup vs baseline: 1.0941x; 1.0055x over previous
"""EntropyGuidance Trainium2 kernel, fp16-I/O Act-roofline variant.

Each core handles 2 samples (B=16 over 8 cores), each sample packed as
[128 partitions = 64 channels x 2 HW-halves (p = 2c+h), 8192 free].

The Activation engine is the roofline here (~31us of exp work that no
other engine can run), so the schedule keeps Act 100% fed:
  - fp16 on both DMA directions (host casts): loads ride Pool/SWDGE
    (text) + SP/HWDGE (vis), stores SP/HWDGE; DMA busy ~35us.
  - PE is warmed with ~8 wide dummy matmuls before the first transpose
    so the p-state ramp never doubles transpose time mid-pipeline.
  - sample 0 leads with two 512-col groups so the first exp starts
    ~1us earlier; the exp->xe->transpose PSUM round-trip is kept under
    2x the exp stage time (2 tv buffers).
  - phase-2 Act ops are emitted AFTER the next sample's first exp so a
    blocked Ln never head-of-line blocks the exp stream.
  - out = vis + g*text is split DVE (tensor_scalar_mul 4x + tensor_add
    2x, 0.78 ns/col) / Pool (one-op scalar_tensor_tensor, 1.39 ns/col)
    so the tail after g1 shrinks and DVE never starves the xe products.
"""

import sys

sys.path.insert(0, "/opt/trn_rl_repo")

import math
from contextlib import ExitStack

import numpy as np

import concourse.bacc as bacc
import concourse.tile as tile
from concourse import mybir
from concourse.bass_utils import run_bass_kernel_spmd
from concourse.masks import make_identity

if not hasattr(bacc, "_orig_get_act_tables"):
    bacc._orig_get_act_tables = bacc.get_activation_tables


def _lnexp_only_tables(module_arch):
    tabs = bacc._orig_get_act_tables(module_arch)
    return {
        name: (funcs if name == "natural_log_exp_and_others" else set())
        for name, funcs in tabs.items()
    }


bacc.get_activation_tables = _lnexp_only_tables

F32 = mybir.dt.float32
FP16 = mybir.dt.float16
AF = mybir.ActivationFunctionType
ALU = mybir.AluOpType

B, C, H, W = 16, 64, 128, 128
HW = H * W                      # 16384
HH = HW // 2                    # 8192 per half
NCORES = 8
P = 128                         # partitions = 64 channels x 2 halves
EPS = 1e-9

# per-sample load pieces along the 8192 free axis
PIECES_S = [
    [(0, 512), (512, 512), (1024, 1024), (2048, 2048), (4096, 2048),
     (6144, 2048)],
    [(0, 2048), (2048, 2048), (4096, 2048), (6144, 2048)],
]
# transpose/exp groups (off, width); J blocks per group = width/128.
# sample 1 ends with two 512 groups so the last exp -> J/T-stop chain
# (which gates g1 and the tail stores) is as short as possible
GROUPS_S = [
    [(0, 512), (512, 512)] + [(1024 + g * 1024, 1024) for g in range(7)],
    [(g * 1024, 1024) for g in range(7)] + [(7168, 512), (7680, 512)],
]
# madd work split: (piece, col_lo, col_hi, engine)
# DVE gets 2-op (0.78/col) chunks; sample-0 chunks are 512 wide so an
# in-flight chunk never delays an xe product by more than ~0.4us.
# "pe" chunks run Dg@text + I@vis on the Tensor engine into PSUM with
# an Act PSUM->SBUF fp16 copy (both engines idle at the tail; PE
# chunks are 512 wide since a matmul output must fit one PSUM bank)
MADD_S = [
    # sample 0: 16 x 512-col chunks; "dvepool" = DVE 4x tensor_scalar
    # for g*text then the +vis add on the idle Pool engine, so each
    # group iteration absorbs ~3 chunks without out-pacing the 1.9us
    # Act stage on DVE
    [(0, 0, 512, "dve"), (1, 0, 512, "dve"),
     (2, 0, 512, "dve"), (2, 512, 1024, "dve"),
     (3, 0, 512, "dve"), (3, 512, 1024, "dve"),
     (3, 1024, 1536, "dvepool"), (3, 1536, 2048, "dvepool"),
     (4, 0, 512, "dvepool"), (4, 512, 1024, "dvepool"),
     (4, 1024, 1536, "dvepool"), (4, 1536, 2048, "dvepool"),
     (5, 0, 512, "dvepool"), (5, 512, 1024, "dvepool"),
     (5, 1024, 1536, "dvepool"), (5, 1536, 2048, "dvepool")],
    # sample 1 (tail): DVE 5120 cols, PE+Act 3072 cols in parallel
    [(0, 0, 1024, "dve"), (0, 1024, 2048, "dve"),
     (1, 0, 1024, "dve"), (1, 1024, 2048, "dve"),
     (2, 0, 1024, "dve"),
     (2, 1024, 1536, "pe"), (2, 1536, 2048, "pe"),
     (3, 0, 512, "pe"), (3, 512, 1024, "pe"),
     (3, 1024, 1536, "pe"), (3, 1536, 2048, "pe")],
]


def _grp_src(pieces, groups):
    out = []
    for off, w in groups:
        for pi, (o, pw) in enumerate(pieces):
            if o <= off and off + w <= o + pw:
                out.append((pi, off - o))
                break
        else:
            raise AssertionError((off, w))
    return out


GRP_SRC_S = [_grp_src(PIECES_S[s], GROUPS_S[s]) for s in range(2)]
NBLK = 64                       # J blocks per sample
# Ln(joint) rescale: raw J entries are ~Sv*St/HW ~ 4.5e4, so scale the act
# Ln input to ~1.4 and add ln(HW^2 / JSCALE) back via lnSv
JSCALE = 1.0 / 32768.0
LNK0 = math.log(float(HW) * float(HW) / JSCALE)


def _build_program():
    nc = bacc.Bacc()
    vis_d = nc.declare_dram_parameter("vis", [2, 2, C, HH], FP16,
                                      isOutput=False)
    text_d = nc.declare_dram_parameter("text", [2, 2, C, HH], FP16,
                                       isOutput=False)
    out_d = nc.declare_dram_parameter("out", [2, 2, C, HH], FP16,
                                      isOutput=True)

    with ExitStack() as ctx:
        tc = ctx.enter_context(tile.TileContext(nc))
        _emit(ctx, tc, vis_d, text_d, out_d)
    nc.finalize()
    return nc


def _emit(ctx: ExitStack, tc: tile.TileContext, vis_d, text_d, out_d):
    nc = tc.nc

    io = ctx.enter_context(tc.tile_pool(name="io", bufs=2))
    etvp = ctx.enter_context(tc.tile_pool(name="etv", bufs=2))
    xep = ctx.enter_context(tc.tile_pool(name="xe", bufs=2))
    outp = ctx.enter_context(tc.tile_pool(name="outp", bufs=2))
    consts = ctx.enter_context(tc.tile_pool(name="consts", bufs=1))
    small = ctx.enter_context(tc.tile_pool(name="small", bufs=2))
    # PSUM budget (8 banks): tv 3 bufs x 2 banks = 6, plus ONE bank
    # holding both samples' J/stat accumulators AND every phase-2
    # matmul output AND the warm-up target, packed as column ranges of
    # a single [P, 512] f32 tile.  The third tv buffer gives the
    # exp->xe->transpose round-trip enough slack that the Act exp
    # stream (the roofline) never stalls on it.
    tvps = ctx.enter_context(tc.tile_pool(name="tvps", bufs=3, space="PSUM"))
    jst = ctx.enter_context(tc.tile_pool(name="jst", bufs=1, space="PSUM"))

    tsb = {}   # (s, piece) -> text fp16 tile
    vsb = {}
    jtl = {}   # s -> [P, 132] f32 PSUM: J (0:128) + S_v/S_t/T (128:131)
    tvl = {}   # (s, g) -> transpose-group PSUM tile
    etl = {}   # (s, g) -> exp(group) SBUF tile
    xel = {}   # (s, g) -> t*e^t group tile

    # fp16 identity for the 128x128 PE transposes
    ident_h = consts.tile([P, P], FP16)

    def emit_loads(s, with_ident=False):
        # ALL loads ride the single Pool/SWDGE queue: transfers then hit
        # the (exclusive) DMA device in exactly emission order
        # (t0,v0,t1,v1,...), so each transpose group's pair lands
        # together and the exp stream is paced by deliveries, never by
        # cross-queue arbitration (HWDGE queues would otherwise race
        # ahead with vis/s1 pieces and starve the s0 text stream).
        for pi, (o, w) in enumerate(PIECES_S[s]):
            t = io.tile([P, w], FP16, tag=f"t{pi}", name=f"t{s}_{pi}")
            v = io.tile([P, w], FP16, tag=f"v{pi}", name=f"v{s}_{pi}")
            tsb[(s, pi)] = t
            vsb[(s, pi)] = v
            src_t = text_d[s, :, :, o:o + w].rearrange("h c n -> (h c) n")
            nc.gpsimd.dma_start(out=t, in_=src_t)
            src_v = vis_d[s, :, :, o:o + w].rearrange("h c n -> (h c) n")
            nc.gpsimd.dma_start(out=v, in_=src_v)
            if with_ident and pi == 0:
                # identity lands on the Pool queue right after the first
                # piece pair so transposes aren't blocked behind the
                # whole load stream
                make_identity(nc, ident_h)

    def emit_transposes(s, g):
        off, w = GROUPS_S[s][g]
        pi, lo = GRP_SRC_S[s][g]
        t_src, v_src = tsb[(s, pi)], vsb[(s, pi)]
        nb = w // 128
        tv = tvps.tile([P, 2 * w], FP16, tag="tv", name=f"tv{s}_{g}")
        tvl[(s, g)] = tv
        for b in range(nb):
            c0 = lo + b * 128
            nc.tensor.transpose(tv[:, b * 128:(b + 1) * 128],
                                t_src[:, c0:c0 + 128], ident_h)
        for b in range(nb):
            c0 = lo + b * 128
            nc.tensor.transpose(tv[:, w + b * 128:w + (b + 1) * 128],
                                v_src[:, c0:c0 + 128], ident_h)

    def emit_exp_xe(s, g, split=False):
        off, w = GROUPS_S[s][g]
        tv = tvl[(s, g)]
        etv = etvp.tile([P, 2 * w], FP16, tag="etv", name=f"etv{s}_{g}")
        etl[(s, g)] = etv
        if split:
            # text half first so xe (and the J rhs) unblock earlier
            nc.scalar.activation(out=etv[:, 0:w], in_=tv[:, 0:w],
                                 func=AF.Exp)
            nc.scalar.activation(out=etv[:, w:2 * w], in_=tv[:, w:2 * w],
                                 func=AF.Exp)
        else:
            nc.scalar.activation(out=etv, in_=tv, func=AF.Exp)
        xe = xep.tile([P, w], FP16, tag="xe", name=f"xe{s}_{g}")
        xel[(s, g)] = xe
        # tensor_tensor gets the 2x DVE mode; this read also releases tv
        nc.vector.tensor_mul(xe, tv[:, 0:w], etv[:, 0:w])

    def _blk0(s, g):
        return GROUPS_S[s][g][0] // 128

    def emit_matmuls(s, g):
        off, w = GROUPS_S[s][g]
        etv, j_t = etl[(s, g)], jtl[s]
        for b in range(w // 128):
            k = _blk0(s, g) + b
            sp_f = (k == NBLK - 1)
            etT = etv[:, b * 128:(b + 1) * 128]
            evT = etv[:, w + b * 128:w + (b + 1) * 128]
            # all chains accumulate onto PSUM zeros with start=False;
            # lhsT=etT so JJ[pt, pv] rows are text-side (the d index
            # the phase-2 rst row-scale and d-contraction need)
            nc.tensor.matmul(j_t[:, 0:128], lhsT=etT, rhs=evT,
                             start=False, stop=sp_f,
                             skip_group_check=True)
            nc.tensor.matmul(j_t[:, 128:129], lhsT=evT, rhs=ones128h,
                             start=False, stop=sp_f,
                             skip_group_check=True)
            nc.tensor.matmul(j_t[:, 129:130], lhsT=etT, rhs=ones128h,
                             start=False, stop=sp_f,
                             skip_group_check=True)

    def emit_tstats(s, g):
        # T = sum_n t*e^t column sums; emitted one group late so the PE
        # queue never blocks on the DVE xe product at the queue head
        off, w = GROUPS_S[s][g]
        xe, j_t = xel[(s, g)], jtl[s]
        for b in range(w // 128):
            k = _blk0(s, g) + b
            nc.tensor.matmul(j_t[:, 130:131],
                             lhsT=xe[:, b * 128:(b + 1) * 128],
                             rhs=ones128h, start=False,
                             stop=(k == NBLK - 1),
                             skip_group_check=True)

    def emit_phase2_stats(s):
        """negent/recips branch; independent of the J merge. Reads the
        Sv/St/T stat columns straight out of PSUM where possible."""
        # h-fold via a partition-strided DVE add straight out of PSUM:
        # sums[c] = stats[2c] + stats[2c+1], one op, no PE round-trip
        sh = small.tile([C, 3], F32, tag="sumh", name=f"sumh{s}")
        nc.vector.tensor_copy(out=sh, in_=jtl[s][C:P, 128:131])
        sums = small.tile([C, 3], F32, tag="sums", name=f"sums{s}")
        nc.vector.tensor_add(sums, sh, jtl[s][0:C, 128:131])
        recips = small.tile([C, 3], F32, tag="recips", name=f"recips{s}")
        nc.vector.reciprocal(out=recips[:, 0:2], in_=sums[:, 0:2])
        rst = recips[:, 1:2]
        nc.vector.tensor_scalar_mul(out=recips[:, 2:3], in0=recips[:, 0:1],
                                    scalar1=0.5)
        lnls = small.tile([C, 2], F32, tag="lnls", name=f"lnls{s}")
        nc.scalar.activation(out=lnls, in_=sums[:, 0:2], func=AF.Ln)
        # fold the lt0 rescale constant ln(HW^2 / JSCALE) into lnSv so the
        # Ln of the raw joint runs on O(1) inputs (act table accuracy)
        nc.vector.tensor_scalar_add(out=lnls[:, 0:1], in0=lnls[:, 0:1],
                                    scalar1=-LNK0)
        # negent = T/St - lnSt; bias for the final Exp folds negent in:
        # biasv = -negent - (1 + HW*EPS)
        negent = small.tile([C, 1], F32, tag="negent", name=f"negent{s}")
        nc.vector.scalar_tensor_tensor(
            out=negent, in0=sums[:, 2:3], scalar=rst, in1=lnls[:, 1:2],
            op0=ALU.mult, op1=ALU.subtract)
        biasv = small.tile([C, 1], F32, tag="biasv", name=f"biasv{s}")
        nc.vector.tensor_scalar(out=biasv, in0=negent, scalar1=-1.0,
                                scalar2=-(1.0 + HW * EPS), op0=ALU.mult,
                                op1=ALU.add)
        r2 = small.tile([C, 2], F32, tag="r2", name=f"r2{s}")
        nc.vector.memset(r2[:, 0:1], 1.0)
        nc.vector.tensor_copy(out=r2[:, 1:2], in_=lnls[:, 1:2])
        return recips, lnls, biasv, r2

    def emit_phase2_ji(s):
        """J-branch head: only needs the J matmul chain stop (emitted
        before the T-stat stop so it starts ~0.5us earlier).
        J64T[d,c] = sum_h JJ[(c,h),(d,h)]"""
        jh = small.tile([C, C], F32, tag="jjh", name=f"jjh{s}")
        nc.vector.tensor_copy(out=jh, in_=jtl[s][C:P, C:P])
        j64sum = small.tile([C, C], F32, tag="jj", name=f"jj{s}")
        nc.vector.tensor_add(j64sum, jh, jtl[s][0:C, 0:C])
        return j64sum

    def emit_phase2_lt0(s, j64sum):
        lt0 = small.tile([C, C], F32, tag="lt0", name=f"lt0{s}")
        nc.scalar.activation(out=lt0, in_=j64sum, func=AF.Ln,
                             scale=JSCALE)
        return lt0

    def emit_phase2_mi(s, stats, j64sum, lt0):
        """mi via the separable log:
        ln(HW^2*J/(Sv*St)) = ln(HW^2*Jraw) - lnSv_c - lnSt_d
        (the +EPS inside the reference log shifts values ~1e-9; dropped)
        """
        recips, lnls, biasv, r2 = stats
        rsv05 = recips[:, 2:3]
        rst = recips[:, 1:2]
        lnsv = lnls[:, 0:1]
        # PSUM->SBUF copy doubles as the rst_d row scaling
        j64t = small.tile([C, C], F32, tag="j64t_sb", name=f"j64t_sb{s}")
        nc.vector.tensor_scalar_mul(out=j64t, in0=j64sum, scalar1=rst)
        q = small.tile([C, C], F32, tag="q", name=f"q{s}")
        nc.vector.tensor_mul(q, lt0, j64t)
        ry_ps = jt_all[0:C, 459:461]
        nc.tensor.matmul(ry_ps, lhsT=j64t, rhs=r2, start=True, stop=True)
        u1_ps = jt_all[0:C, 461:462]
        nc.tensor.matmul(u1_ps, lhsT=q, rhs=ones64[:, 0:1], start=True,
                         stop=True)
        # z1 = ry0*lnsv + ry1 ; zz = (u1 - z1) * 0.5/Sv  (two-scalar ops)
        z1 = small.tile([C, 1], F32, tag="z1", name=f"z1{s}")
        nc.vector.tensor_scalar(out=z1, in0=ry_ps[:, 0:1], scalar1=lnsv,
                                scalar2=ry_ps[:, 1:2], op0=ALU.mult,
                                op1=ALU.add)
        zz = small.tile([C, 1], F32, tag="zz", name=f"zz{s}")
        nc.vector.tensor_scalar(out=zz, in0=u1_ps, scalar1=z1,
                                scalar2=rsv05, op0=ALU.subtract,
                                op1=ALU.mult)
        mib_ps = jt_all[0:C, 462:463]
        nc.tensor.matmul(mib_ps, lhsT=ones64, rhs=zz, start=True,
                         stop=True)
        return mib_ps

    def emit_phase2_g(s, mib_ps, stats):
        biasv = stats[2]
        g64 = small.tile([C, 1], F32, tag="g64", name=f"g64{s}")
        nc.scalar.activation(out=g64, in_=mib_ps, func=AF.Exp, scale=-1.0,
                             bias=biasv)
        nc.vector.tensor_scalar_add(out=g64, in0=g64, scalar1=1.0)
        nc.vector.reciprocal(out=g64, in_=g64)
        g = small.tile([P, 1], F32, tag="g", name=f"g{s}")
        nc.vector.tensor_copy(out=g[0:C, :], in_=g64)
        nc.vector.tensor_copy(out=g[C:P, :], in_=g64)
        return g, g64

    otl = {}

    def emit_madd(s, g, ci, dg=None):
        pi, lo, hi, eng = MADD_S[s][ci]
        o, w = PIECES_S[s][pi]
        if (s, pi) not in otl:
            otl[(s, pi)] = outp.tile([P, w], FP16, tag=f"o{s}_{pi}",
                                     name=f"o{s}_{pi}")
        ot = otl[(s, pi)]
        if eng == "dve" or eng == "dvepool":
            # g*text at 4x (f32 ptr scalar exempt from the 2-byte rule),
            # then += vis at 2x (or on the idle Pool for dvepool)
            nc.vector.tensor_scalar_mul(out=ot[:, lo:hi],
                                        in0=tsb[(s, pi)][:, lo:hi],
                                        scalar1=g)
            add_eng = nc.vector if eng == "dve" else nc.gpsimd
            add_eng.tensor_add(ot[:, lo:hi], ot[:, lo:hi],
                               vsb[(s, pi)][:, lo:hi])
        else:
            # Dg@text + I@vis accumulated in PSUM, Act copies to fp16
            pm = tvps.tile([P, hi - lo], F32, tag="tv",
                           name=f"pm{s}_{ci}")
            nc.tensor.matmul(pm, lhsT=dg, rhs=tsb[(s, pi)][:, lo:hi],
                             start=True, stop=False)
            nc.tensor.matmul(pm, lhsT=ident_h, rhs=vsb[(s, pi)][:, lo:hi],
                             start=False, stop=True)
            nc.scalar.activation(out=ot[:, lo:hi], in_=pm, func=AF.Copy)

    store_ctr = [0]

    def emit_store(s, pi, lo=None, hi=None):
        # alternate SP/Pool queues so store issue never serializes on
        # one sequencer at the tail; lo/hi store a piece sub-range so
        # tail halves stream out as soon as their chunks finish
        o, w = PIECES_S[s][pi]
        if lo is None:
            lo, hi = 0, w
        dst = out_d[s, :, :, o + lo:o + hi].rearrange("h c n -> (h c) n")
        # sample-0 stores always ride SP: the Pool queue is busy with
        # sample-1 load preps and the Pool-assisted madd adds mid-kernel
        if s == 0:
            eng = nc.sync
        else:
            eng = nc.sync if store_ctr[0] % 2 == 0 else nc.gpsimd
            store_ctr[0] += 1
        eng.dma_start(out=dst, in_=otl[(s, pi)][:, lo:hi])

    # ---- emission ----
    # single-bank PSUM mega-tile: j0 0:132, j1 132:264, warm 264:392,
    # j64t 392:456, sums 456:459, ry 459:461, u1 461, mib 462, gbc 463
    jt_all = jst.tile([P, 512], F32, tag="jall", name="jall")
    for s in range(2):
        jtl[s] = jt_all[:, s * 132:(s + 1) * 132]

    # DVE-built constants + PE warm-up fodder (DVE is idle at t=0)
    ones128h = consts.tile([P, 1], FP16)
    nc.vector.memset(ones128h, 1.0)
    junk = consts.tile([P, 192], FP16)
    nc.vector.memset(junk, 0.0)

    emit_loads(0, with_ident=True)

    # nudge the PE p-state ramp before the first transposes (engine
    # init means PE can't start before ~2.4us; the first piece lands
    # ~2.9us, so just a few warms to leave the lowest p-state)
    warm_ps = jt_all[0:1, 264:392]
    for i in range(3):
        nc.tensor.matmul(warm_ps, lhsT=ones128h, rhs=junk[:, 0:128],
                         start=True, stop=True)

    ones64 = consts.tile([C, C], F32)
    nc.gpsimd.memset(ones64, 1.0)

    NG0 = len(GROUPS_S[0])
    NG1 = len(GROUPS_S[1])

    # sample 0, software-pipelined: transposes TWO groups ahead of the
    # exp (3 tv buffers), so group g's J matmuls waiting on exp(g) at
    # the PE queue head never block the transposes of group g+2;
    # T-stat matmuls one group behind (they wait on DVE xe)
    emit_transposes(0, 0)
    emit_transposes(0, 1)
    emit_transposes(0, 2)
    emit_exp_xe(0, 0, split=True)
    emit_matmuls(0, 0)
    for g in range(1, NG0 - 2):
        emit_transposes(0, g + 2)
        emit_exp_xe(0, g)
        emit_matmuls(0, g)
        emit_tstats(0, g - 1)
    emit_exp_xe(0, NG0 - 2)
    emit_matmuls(0, NG0 - 2)
    emit_tstats(0, NG0 - 3)
    emit_exp_xe(0, NG0 - 1)
    emit_matmuls(0, NG0 - 1)
    ji0 = emit_phase2_ji(0)       # J branch head right at J-stop
    emit_tstats(0, NG0 - 2)
    emit_tstats(0, NG0 - 1)

    emit_loads(1)

    # sample 1 pipelined; sample-0 phase 2 is emitted AFTER exp(1,0) so
    # its (dependency-blocked) Act ops never head-of-line block the exp
    # stream; its Act ops then slot between sample-1 exps
    emit_transposes(1, 0)
    emit_transposes(1, 1)
    emit_transposes(1, 2)
    emit_exp_xe(1, 0)
    emit_matmuls(1, 0)

    stats0 = emit_phase2_stats(0)
    lt00 = emit_phase2_lt0(0, ji0)
    mib0 = emit_phase2_mi(0, stats0, ji0, lt00)

    emit_transposes(1, 3)
    emit_exp_xe(1, 1)
    emit_matmuls(1, 1)
    emit_tstats(1, 0)

    g0, g64_0 = emit_phase2_g(0, mib0, stats0)

    # s0 madd chunks drip into the group iterations AFTER the xe
    # product; Pool-assisted chunks go first (their Pool adds must
    # clear the Pool queue before phase-2(1) needs it), and the short
    # 512-group iterations at the tail carry at most one chunk
    remaining = {}
    for ci, (pi, lo, hi, eng) in enumerate(MADD_S[0]):
        remaining[pi] = remaining.get(pi, 0) + 1
    DRIP_PLAN = [[8, 9, 10, 14], [11, 12, 13, 15], [6, 7, 0], [1, 2, 3], [4, 5]]

    def drip(cis):
        for ci in cis:
            emit_madd(0, g0, ci)
            pi = MADD_S[0][ci][0]
            remaining[pi] -= 1
            if remaining[pi] == 0:
                emit_store(0, pi)

    for g in range(2, NG1 - 2):
        emit_transposes(1, g + 2)
        emit_exp_xe(1, g)
        emit_matmuls(1, g)
        emit_tstats(1, g - 1)
        drip(DRIP_PLAN[g - 2])
    emit_exp_xe(1, NG1 - 2)
    emit_matmuls(1, NG1 - 2)
    emit_tstats(1, NG1 - 3)
    emit_exp_xe(1, NG1 - 1)
    emit_matmuls(1, NG1 - 1)
    ji1 = emit_phase2_ji(1)       # J branch head right at J-stop
    emit_tstats(1, NG1 - 2)
    emit_tstats(1, NG1 - 1)

    stats1 = emit_phase2_stats(1)
    lt01 = emit_phase2_lt0(1, ji1)
    mib1 = emit_phase2_mi(1, stats1, ji1, lt01)
    g1, g64_1 = emit_phase2_g(1, mib1, stats1)

    # tail: PE+Act chunks and DVE chunks run in parallel; each 1024-col
    # half is stored the moment its chunks are written.  Dg = diag(g1)
    # feeds the PE path (4x tensor_scalar off ident)
    dg1 = small.tile([P, P], FP16, tag="dg", name="dg1")
    nc.vector.tensor_scalar_mul(out=dg1, in0=ident_h, scalar1=g1)
    for ci in (5, 6):                 # pe: p2[1024:2048]
        emit_madd(1, g1, ci, dg=dg1)
    emit_madd(1, g1, 4)               # dve: p2[0:1024]
    emit_store(1, 2, 0, 1024)
    emit_store(1, 2, 1024, 2048)
    for ci in (7, 8):                 # pe: p3[0:1024]
        emit_madd(1, g1, ci, dg=dg1)
    emit_madd(1, g1, 0)               # dve: p0[0:1024]
    emit_store(1, 3, 0, 1024)
    emit_store(1, 0, 0, 1024)
    for ci in (9, 10):                # pe: p3[1024:2048]
        emit_madd(1, g1, ci, dg=dg1)
    emit_madd(1, g1, 1)               # dve: p0[1024:2048]
    emit_store(1, 3, 1024, 2048)
    emit_store(1, 0, 1024, 2048)
    emit_madd(1, g1, 2)               # dve: p1[0:1024]
    emit_store(1, 1, 0, 1024)
    emit_madd(1, g1, 3)               # dve: p1[1024:2048]
    emit_store(1, 1, 1024, 2048)


_PROGRAM = None


def _get_program():
    global _PROGRAM
    if _PROGRAM is None:
        _PROGRAM = _build_program()
    return _PROGRAM


def kernel(vis_feat: np.ndarray, text_feat: np.ndarray) -> np.ndarray:
    nc = _get_program()
    vis = np.ascontiguousarray(vis_feat, dtype=np.float16)
    text = np.ascontiguousarray(text_feat, dtype=np.float16)
    bpc = B // NCORES
    in_maps = [
        {
            "vis": np.ascontiguousarray(
                vis[i * bpc:(i + 1) * bpc].reshape(bpc, C, 2, HH)
                .transpose(0, 2, 1, 3)),
            "text": np.ascontiguousarray(
                text[i * bpc:(i + 1) * bpc].reshape(bpc, C, 2, HH)
                .transpose(0, 2, 1, 3)),
        }
        for i in range(NCORES)
    ]
    res = run_bass_kernel_spmd(nc, in_maps, list(range(NCORES)))
    out = np.concatenate(
        [np.asarray(r["out"]).reshape(bpc, 2, C, HH).transpose(0, 2, 1, 3)
         .reshape(bpc, C, H, W) for r in res.results],
        axis=0)
    return out.astype(np.float32)


# revision 37
# speedup vs baseline: 1.1202x; 1.0238x over previous
"""EntropyGuidance Trainium2 kernel, fp16-I/O Act-roofline variant.

Each core handles 2 samples (B=16 over 8 cores), each sample packed as
[128 partitions = 64 channels x 2 HW-halves (p = 2c+h), 8192 free].

The Activation engine is the roofline here (~31us of exp work that no
other engine can run), so the schedule keeps Act 100% fed:
  - fp16 on both DMA directions (host casts): loads ride Pool/SWDGE
    (text) + SP/HWDGE (vis), stores SP/HWDGE; DMA busy ~35us.
  - PE is warmed with ~8 wide dummy matmuls before the first transpose
    so the p-state ramp never doubles transpose time mid-pipeline.
  - sample 0 leads with two 512-col groups so the first exp starts
    ~1us earlier; the exp->xe->transpose PSUM round-trip is kept under
    2x the exp stage time (2 tv buffers).
  - phase-2 Act ops are emitted AFTER the next sample's first exp so a
    blocked Ln never head-of-line blocks the exp stream.
  - out = vis + g*text is split DVE (tensor_scalar_mul 4x + tensor_add
    2x, 0.78 ns/col) / Pool (one-op scalar_tensor_tensor, 1.39 ns/col)
    so the tail after g1 shrinks and DVE never starves the xe products.
"""

import sys

sys.path.insert(0, "/opt/trn_rl_repo")

import math
from contextlib import ExitStack

import numpy as np

import concourse.bacc as bacc
import concourse.tile as tile
from concourse import mybir
from concourse.bass_utils import run_bass_kernel_spmd
from concourse.masks import make_identity

if not hasattr(bacc, "_orig_get_act_tables"):
    bacc._orig_get_act_tables = bacc.get_activation_tables


def _lnexp_only_tables(module_arch):
    tabs = bacc._orig_get_act_tables(module_arch)
    return {
        name: (funcs if name == "natural_log_exp_and_others" else set())
        for name, funcs in tabs.items()
    }


bacc.get_activation_tables = _lnexp_only_tables

F32 = mybir.dt.float32
FP16 = mybir.dt.float16
AF = mybir.ActivationFunctionType
ALU = mybir.AluOpType

B, C, H, W = 16, 64, 128, 128
HW = H * W                      # 16384
HH = HW // 2                    # 8192 per half
NCORES = 8
P = 128                         # partitions = 64 channels x 2 halves
EPS = 1e-9

# per-sample load pieces along the 8192 free axis
PIECES_S = [
    [(0, 512), (512, 512), (1024, 1024), (2048, 2048), (4096, 2048),
     (6144, 2048)],
    [(0, 2048), (2048, 2048), (4096, 2048), (6144, 2048)],
]
# transpose/exp groups (off, width); J blocks per group = width/128.
# sample 1 ends with two 512 groups so the last exp -> J/T-stop chain
# (which gates g1 and the tail stores) is as short as possible
GROUPS_S = [
    [(0, 512), (512, 512)] + [(1024 + g * 1024, 1024) for g in range(7)],
    [(g * 1024, 1024) for g in range(7)] + [(7168, 512), (7680, 512)],
]
# madd work split: (piece, col_lo, col_hi, engine)
# DVE gets 2-op (0.78/col) chunks; sample-0 chunks are 512 wide so an
# in-flight chunk never delays an xe product by more than ~0.4us.
# "pe" chunks run Dg@text + I@vis on the Tensor engine into PSUM with
# an Act PSUM->SBUF fp16 copy (both engines idle at the tail; PE
# chunks are 512 wide since a matmul output must fit one PSUM bank)
MADD_S = [
    # sample 0: 16 x 512-col chunks; "dvepool" = DVE 4x tensor_scalar
    # for g*text then the +vis add on the idle Pool engine, so each
    # group iteration absorbs ~3 chunks without out-pacing the 1.9us
    # Act stage on DVE
    [(0, 0, 512, "dve"), (1, 0, 512, "dve"),
     (2, 0, 512, "dve"), (2, 512, 1024, "dve"),
     (3, 0, 512, "dve"), (3, 512, 1024, "dve"),
     (3, 1024, 1536, "dvepool"), (3, 1536, 2048, "dvepool"),
     (4, 0, 512, "dvepool"), (4, 512, 1024, "dvepool"),
     (4, 1024, 1536, "dvepool"), (4, 1536, 2048, "dvepool"),
     (5, 0, 512, "dvepool"), (5, 512, 1024, "dvepool"),
     (5, 1024, 1536, "dvepool"), (5, 1536, 2048, "dvepool")],
    # sample 1 (tail): DVE 5120 cols, PE+Act 3072 cols in parallel
    [(0, 0, 1024, "dve"), (0, 1024, 2048, "dve"),
     (1, 0, 1024, "dve"), (1, 1024, 2048, "dve"),
     (2, 0, 1024, "dve"),
     (2, 1024, 1536, "pe"), (2, 1536, 2048, "pe"),
     (3, 0, 512, "pe"), (3, 512, 1024, "pe"),
     (3, 1024, 1536, "pe"), (3, 1536, 2048, "pe")],
]


def _grp_src(pieces, groups):
    out = []
    for off, w in groups:
        for pi, (o, pw) in enumerate(pieces):
            if o <= off and off + w <= o + pw:
                out.append((pi, off - o))
                break
        else:
            raise AssertionError((off, w))
    return out


GRP_SRC_S = [_grp_src(PIECES_S[s], GROUPS_S[s]) for s in range(2)]
NBLK = 64                       # J blocks per sample
# Ln(joint) rescale: raw J entries are ~Sv*St/HW ~ 4.5e4, so scale the act
# Ln input to ~1.4 and add ln(HW^2 / JSCALE) back via lnSv
JSCALE = 1.0 / 32768.0
LNK0 = math.log(float(HW) * float(HW) / JSCALE)


def _build_program():
    nc = bacc.Bacc()
    vis_d = nc.declare_dram_parameter("vis", [2, 2, C, HH], FP16,
                                      isOutput=False)
    text_d = nc.declare_dram_parameter("text", [2, 2, C, HH], FP16,
                                       isOutput=False)
    out_d = nc.declare_dram_parameter("out", [2, 2, C, HH], FP16,
                                      isOutput=True)

    with ExitStack() as ctx:
        tc = ctx.enter_context(tile.TileContext(nc))
        _emit(ctx, tc, vis_d, text_d, out_d)
    nc.finalize()
    return nc


def _emit(ctx: ExitStack, tc: tile.TileContext, vis_d, text_d, out_d):
    nc = tc.nc

    io = ctx.enter_context(tc.tile_pool(name="io", bufs=2))
    etvp = ctx.enter_context(tc.tile_pool(name="etv", bufs=2))
    xep = ctx.enter_context(tc.tile_pool(name="xe", bufs=2))
    outp = ctx.enter_context(tc.tile_pool(name="outp", bufs=2))
    consts = ctx.enter_context(tc.tile_pool(name="consts", bufs=1))
    small = ctx.enter_context(tc.tile_pool(name="small", bufs=2))
    # PSUM budget (8 banks): tv 3 bufs x 2 banks = 6, plus ONE bank
    # holding both samples' J/stat accumulators AND every phase-2
    # matmul output AND the warm-up target, packed as column ranges of
    # a single [P, 512] f32 tile.  The third tv buffer gives the
    # exp->xe->transpose round-trip enough slack that the Act exp
    # stream (the roofline) never stalls on it.
    tvps = ctx.enter_context(tc.tile_pool(name="tvps", bufs=3, space="PSUM"))
    jst = ctx.enter_context(tc.tile_pool(name="jst", bufs=1, space="PSUM"))

    tsb = {}   # (s, piece) -> text fp16 tile
    vsb = {}
    jtl = {}   # s -> [P, 132] f32 PSUM: J (0:128) + S_v/S_t/T (128:131)
    tvl = {}   # (s, g) -> transpose-group PSUM tile
    etl = {}   # (s, g) -> exp(group) SBUF tile
    xel = {}   # (s, g) -> t*e^t group tile

    # fp16 identity for the 128x128 PE transposes
    ident_h = consts.tile([P, P], FP16)

    def emit_loads(s, with_ident=False):
        # ALL loads ride the single Pool/SWDGE queue: transfers then hit
        # the (exclusive) DMA device in exactly emission order
        # (t0,v0,t1,v1,...), so each transpose group's pair lands
        # together and the exp stream is paced by deliveries, never by
        # cross-queue arbitration (HWDGE queues would otherwise race
        # ahead with vis/s1 pieces and starve the s0 text stream).
        for pi, (o, w) in enumerate(PIECES_S[s]):
            t = io.tile([P, w], FP16, tag=f"t{pi}", name=f"t{s}_{pi}")
            v = io.tile([P, w], FP16, tag=f"v{pi}", name=f"v{s}_{pi}")
            tsb[(s, pi)] = t
            vsb[(s, pi)] = v
            src_t = text_d[s, :, :, o:o + w].rearrange("h c n -> (h c) n")
            src_v = vis_d[s, :, :, o:o + w].rearrange("h c n -> (h c) n")
            if s == 0:
                # sample-0 text pieces ride SP/HWDGE: they all land
                # early (the exp split runs the text half first), while
                # the vis stream paces with the Pool preps.  Sample 1
                # stays entirely on Pool so it can never overtake
                # sample 0's stream at the DMA device.
                nc.sync.dma_start(out=t, in_=src_t)
            else:
                nc.gpsimd.dma_start(out=t, in_=src_t)
            nc.gpsimd.dma_start(out=v, in_=src_v)
            if with_ident and pi == 0:
                # identity lands on the Pool queue right after the first
                # piece pair so transposes aren't blocked behind the
                # whole load stream
                make_identity(nc, ident_h)

    def emit_transposes(s, g):
        off, w = GROUPS_S[s][g]
        pi, lo = GRP_SRC_S[s][g]
        t_src, v_src = tsb[(s, pi)], vsb[(s, pi)]
        nb = w // 128
        tv = tvps.tile([P, 2 * w], FP16, tag="tv", name=f"tv{s}_{g}")
        tvl[(s, g)] = tv
        for b in range(nb):
            c0 = lo + b * 128
            nc.tensor.transpose(tv[:, b * 128:(b + 1) * 128],
                                t_src[:, c0:c0 + 128], ident_h)
        for b in range(nb):
            c0 = lo + b * 128
            nc.tensor.transpose(tv[:, w + b * 128:w + (b + 1) * 128],
                                v_src[:, c0:c0 + 128], ident_h)

    def emit_exp_xe(s, g, split=False):
        off, w = GROUPS_S[s][g]
        tv = tvl[(s, g)]
        etv = etvp.tile([P, 2 * w], FP16, tag="etv", name=f"etv{s}_{g}")
        etl[(s, g)] = etv
        if split:
            # text half first so xe (and the J rhs) unblock earlier
            nc.scalar.activation(out=etv[:, 0:w], in_=tv[:, 0:w],
                                 func=AF.Exp)
            nc.scalar.activation(out=etv[:, w:2 * w], in_=tv[:, w:2 * w],
                                 func=AF.Exp)
        else:
            nc.scalar.activation(out=etv, in_=tv, func=AF.Exp)
        xe = xep.tile([P, w], FP16, tag="xe", name=f"xe{s}_{g}")
        xel[(s, g)] = xe
        # tensor_tensor gets the 2x DVE mode; this read also releases tv
        nc.vector.tensor_mul(xe, tv[:, 0:w], etv[:, 0:w])

    def _blk0(s, g):
        return GROUPS_S[s][g][0] // 128

    def emit_matmuls(s, g):
        off, w = GROUPS_S[s][g]
        etv, j_t = etl[(s, g)], jtl[s]
        for b in range(w // 128):
            k = _blk0(s, g) + b
            sp_f = (k == NBLK - 1)
            etT = etv[:, b * 128:(b + 1) * 128]
            evT = etv[:, w + b * 128:w + (b + 1) * 128]
            # all chains accumulate onto PSUM zeros with start=False;
            # lhsT=etT so JJ[pt, pv] rows are text-side (the d index
            # the phase-2 rst row-scale and d-contraction need)
            nc.tensor.matmul(j_t[:, 0:128], lhsT=etT, rhs=evT,
                             start=False, stop=sp_f,
                             skip_group_check=True)
            nc.tensor.matmul(j_t[:, 128:129], lhsT=evT, rhs=ones128h,
                             start=False, stop=sp_f,
                             skip_group_check=True)
            nc.tensor.matmul(j_t[:, 129:130], lhsT=etT, rhs=ones128h,
                             start=False, stop=sp_f,
                             skip_group_check=True)

    def emit_tstats(s, g):
        # T = sum_n t*e^t column sums; emitted one group late so the PE
        # queue never blocks on the DVE xe product at the queue head
        off, w = GROUPS_S[s][g]
        xe, j_t = xel[(s, g)], jtl[s]
        for b in range(w // 128):
            k = _blk0(s, g) + b
            nc.tensor.matmul(j_t[:, 130:131],
                             lhsT=xe[:, b * 128:(b + 1) * 128],
                             rhs=ones128h, start=False,
                             stop=(k == NBLK - 1),
                             skip_group_check=True)

    def emit_phase2_stats(s):
        """negent/recips branch; independent of the J merge. Reads the
        Sv/St/T stat columns straight out of PSUM where possible."""
        # h-fold via a partition-strided DVE add straight out of PSUM:
        # sums[c] = stats[2c] + stats[2c+1], one op, no PE round-trip
        sh = small.tile([C, 3], F32, tag="sumh", name=f"sumh{s}")
        nc.vector.tensor_copy(out=sh, in_=jtl[s][C:P, 128:131])
        sums = small.tile([C, 3], F32, tag="sums", name=f"sums{s}")
        nc.vector.tensor_add(sums, sh, jtl[s][0:C, 128:131])
        recips = small.tile([C, 3], F32, tag="recips", name=f"recips{s}")
        nc.vector.reciprocal(out=recips[:, 0:2], in_=sums[:, 0:2])
        rst = recips[:, 1:2]
        nc.vector.tensor_scalar_mul(out=recips[:, 2:3], in0=recips[:, 0:1],
                                    scalar1=0.5)
        lnls = small.tile([C, 2], F32, tag="lnls", name=f"lnls{s}")
        nc.scalar.activation(out=lnls, in_=sums[:, 0:2], func=AF.Ln)
        # fold the lt0 rescale constant ln(HW^2 / JSCALE) into lnSv so the
        # Ln of the raw joint runs on O(1) inputs (act table accuracy)
        nc.vector.tensor_scalar_add(out=lnls[:, 0:1], in0=lnls[:, 0:1],
                                    scalar1=-LNK0)
        # negent = T/St - lnSt; bias for the final Exp folds negent in:
        # biasv = -negent - (1 + HW*EPS)
        negent = small.tile([C, 1], F32, tag="negent", name=f"negent{s}")
        nc.vector.scalar_tensor_tensor(
            out=negent, in0=sums[:, 2:3], scalar=rst, in1=lnls[:, 1:2],
            op0=ALU.mult, op1=ALU.subtract)
        biasv = small.tile([C, 1], F32, tag="biasv", name=f"biasv{s}")
        nc.vector.tensor_scalar(out=biasv, in0=negent, scalar1=-1.0,
                                scalar2=-(1.0 + HW * EPS), op0=ALU.mult,
                                op1=ALU.add)
        r2 = small.tile([C, 2], F32, tag="r2", name=f"r2{s}")
        nc.vector.memset(r2[:, 0:1], 1.0)
        nc.vector.tensor_copy(out=r2[:, 1:2], in_=lnls[:, 1:2])
        return recips, lnls, biasv, r2

    def emit_phase2_ji(s):
        """J-branch head: only needs the J matmul chain stop (emitted
        before the T-stat stop so it starts ~0.5us earlier).
        J64T[d,c] = sum_h JJ[(c,h),(d,h)]"""
        jh = small.tile([C, C], F32, tag="jjh", name=f"jjh{s}")
        nc.vector.tensor_copy(out=jh, in_=jtl[s][C:P, C:P])
        j64sum = small.tile([C, C], F32, tag="jj", name=f"jj{s}")
        nc.vector.tensor_add(j64sum, jh, jtl[s][0:C, 0:C])
        return j64sum

    def emit_phase2_lt0(s, j64sum):
        lt0 = small.tile([C, C], F32, tag="lt0", name=f"lt0{s}")
        nc.scalar.activation(out=lt0, in_=j64sum, func=AF.Ln,
                             scale=JSCALE)
        return lt0

    def emit_phase2_mi(s, stats, j64sum, lt0):
        """mi via the separable log:
        ln(HW^2*J/(Sv*St)) = ln(HW^2*Jraw) - lnSv_c - lnSt_d
        (the +EPS inside the reference log shifts values ~1e-9; dropped)
        """
        recips, lnls, biasv, r2 = stats
        rsv05 = recips[:, 2:3]
        rst = recips[:, 1:2]
        lnsv = lnls[:, 0:1]
        # PSUM->SBUF copy doubles as the rst_d row scaling
        j64t = small.tile([C, C], F32, tag="j64t_sb", name=f"j64t_sb{s}")
        nc.vector.tensor_scalar_mul(out=j64t, in0=j64sum, scalar1=rst)
        q = small.tile([C, C], F32, tag="q", name=f"q{s}")
        nc.vector.tensor_mul(q, lt0, j64t)
        ry_ps = jt_all[0:C, 459:461]
        nc.tensor.matmul(ry_ps, lhsT=j64t, rhs=r2, start=True, stop=True)
        u1_ps = jt_all[0:C, 461:462]
        nc.tensor.matmul(u1_ps, lhsT=q, rhs=ones64[:, 0:1], start=True,
                         stop=True)
        # z1 = ry0*lnsv + ry1 ; zz = (u1 - z1) * 0.5/Sv  (two-scalar ops)
        z1 = small.tile([C, 1], F32, tag="z1", name=f"z1{s}")
        nc.vector.tensor_scalar(out=z1, in0=ry_ps[:, 0:1], scalar1=lnsv,
                                scalar2=ry_ps[:, 1:2], op0=ALU.mult,
                                op1=ALU.add)
        zz = small.tile([C, 1], F32, tag="zz", name=f"zz{s}")
        nc.vector.tensor_scalar(out=zz, in0=u1_ps, scalar1=z1,
                                scalar2=rsv05, op0=ALU.subtract,
                                op1=ALU.mult)
        mib_ps = jt_all[0:C, 462:463]
        nc.tensor.matmul(mib_ps, lhsT=ones64, rhs=zz, start=True,
                         stop=True)
        return mib_ps

    def emit_phase2_g(s, mib_ps, stats):
        biasv = stats[2]
        g64 = small.tile([C, 1], F32, tag="g64", name=f"g64{s}")
        nc.scalar.activation(out=g64, in_=mib_ps, func=AF.Exp, scale=-1.0,
                             bias=biasv)
        nc.vector.tensor_scalar_add(out=g64, in0=g64, scalar1=1.0)
        nc.vector.reciprocal(out=g64, in_=g64)
        g = small.tile([P, 1], F32, tag="g", name=f"g{s}")
        nc.vector.tensor_copy(out=g[0:C, :], in_=g64)
        nc.vector.tensor_copy(out=g[C:P, :], in_=g64)
        return g, g64

    otl = {}

    def emit_madd(s, g, ci, dg=None):
        pi, lo, hi, eng = MADD_S[s][ci]
        o, w = PIECES_S[s][pi]
        if (s, pi) not in otl:
            otl[(s, pi)] = outp.tile([P, w], FP16, tag=f"o{s}_{pi}",
                                     name=f"o{s}_{pi}")
        ot = otl[(s, pi)]
        if eng == "dve" or eng == "dvepool":
            # g*text at 4x (f32 ptr scalar exempt from the 2-byte rule),
            # then += vis at 2x (or on the idle Pool for dvepool)
            nc.vector.tensor_scalar_mul(out=ot[:, lo:hi],
                                        in0=tsb[(s, pi)][:, lo:hi],
                                        scalar1=g)
            add_eng = nc.vector if eng == "dve" else nc.gpsimd
            add_eng.tensor_add(ot[:, lo:hi], ot[:, lo:hi],
                               vsb[(s, pi)][:, lo:hi])
        else:
            # Dg@text + I@vis accumulated in PSUM, Act copies to fp16
            pm = tvps.tile([P, hi - lo], F32, tag="tv",
                           name=f"pm{s}_{ci}")
            nc.tensor.matmul(pm, lhsT=dg, rhs=tsb[(s, pi)][:, lo:hi],
                             start=True, stop=False)
            nc.tensor.matmul(pm, lhsT=ident_h, rhs=vsb[(s, pi)][:, lo:hi],
                             start=False, stop=True)
            nc.scalar.activation(out=ot[:, lo:hi], in_=pm, func=AF.Copy)

    store_ctr = [0]

    def emit_store(s, pi, lo=None, hi=None):
        # alternate SP/Pool queues so store issue never serializes on
        # one sequencer at the tail; lo/hi store a piece sub-range so
        # tail halves stream out as soon as their chunks finish
        o, w = PIECES_S[s][pi]
        if lo is None:
            lo, hi = 0, w
        dst = out_d[s, :, :, o + lo:o + hi].rearrange("h c n -> (h c) n")
        # sample-0 stores always ride SP: the Pool queue is busy with
        # sample-1 load preps and the Pool-assisted madd adds mid-kernel
        if s == 0:
            eng = nc.sync
        else:
            eng = nc.sync if store_ctr[0] % 2 == 0 else nc.gpsimd
            store_ctr[0] += 1
        eng.dma_start(out=dst, in_=otl[(s, pi)][:, lo:hi])

    # ---- emission ----
    # single-bank PSUM mega-tile: j0 0:132, j1 132:264, warm 264:392,
    # j64t 392:456, sums 456:459, ry 459:461, u1 461, mib 462, gbc 463
    jt_all = jst.tile([P, 512], F32, tag="jall", name="jall")
    for s in range(2):
        jtl[s] = jt_all[:, s * 132:(s + 1) * 132]

    # DVE-built constants + PE warm-up fodder (DVE is idle at t=0)
    ones128h = consts.tile([P, 1], FP16)
    nc.vector.memset(ones128h, 1.0)
    junk = consts.tile([P, 192], FP16)
    nc.vector.memset(junk, 0.0)

    emit_loads(0, with_ident=True)

    # nudge the PE p-state ramp before the first transposes (engine
    # init means PE can't start before ~2.4us; the first piece lands
    # ~2.9us, so just a few warms to leave the lowest p-state)
    warm_ps = jt_all[0:1, 264:392]
    for i in range(3):
        nc.tensor.matmul(warm_ps, lhsT=ones128h, rhs=junk[:, 0:128],
                         start=True, stop=True)

    ones64 = consts.tile([C, C], F32)
    nc.gpsimd.memset(ones64, 1.0)

    NG0 = len(GROUPS_S[0])
    NG1 = len(GROUPS_S[1])

    # sample 0, software-pipelined: transposes TWO groups ahead of the
    # exp (3 tv buffers), so group g's J matmuls waiting on exp(g) at
    # the PE queue head never block the transposes of group g+2;
    # T-stat matmuls one group behind (they wait on DVE xe)
    emit_transposes(0, 0)
    emit_transposes(0, 1)
    emit_transposes(0, 2)
    emit_exp_xe(0, 0, split=True)
    emit_matmuls(0, 0)
    for g in range(1, NG0 - 2):
        emit_transposes(0, g + 2)
        emit_exp_xe(0, g)
        emit_matmuls(0, g)
        emit_tstats(0, g - 1)
    emit_exp_xe(0, NG0 - 2)
    emit_matmuls(0, NG0 - 2)
    emit_tstats(0, NG0 - 3)
    emit_exp_xe(0, NG0 - 1)
    emit_matmuls(0, NG0 - 1)
    ji0 = emit_phase2_ji(0)       # J branch head right at J-stop
    emit_tstats(0, NG0 - 2)
    emit_tstats(0, NG0 - 1)

    emit_loads(1)

    # sample 1 pipelined; sample-0 phase 2 is emitted AFTER exp(1,0) so
    # its (dependency-blocked) Act ops never head-of-line block the exp
    # stream; its Act ops then slot between sample-1 exps
    emit_transposes(1, 0)
    emit_transposes(1, 1)
    emit_transposes(1, 2)
    emit_exp_xe(1, 0)
    emit_matmuls(1, 0)

    stats0 = emit_phase2_stats(0)
    lt00 = emit_phase2_lt0(0, ji0)
    mib0 = emit_phase2_mi(0, stats0, ji0, lt00)

    emit_transposes(1, 3)
    emit_exp_xe(1, 1)
    emit_matmuls(1, 1)
    emit_tstats(1, 0)

    g0, g64_0 = emit_phase2_g(0, mib0, stats0)

    # s0 madd chunks drip into the group iterations AFTER the xe
    # product; Pool-assisted chunks go first (their Pool adds must
    # clear the Pool queue before phase-2(1) needs it), and the short
    # 512-group iterations at the tail carry at most one chunk
    remaining = {}
    for ci, (pi, lo, hi, eng) in enumerate(MADD_S[0]):
        remaining[pi] = remaining.get(pi, 0) + 1
    DRIP_PLAN = [[8, 9, 10, 14], [11, 12, 13, 15], [6, 7, 0], [1, 2, 3], [4, 5]]

    def drip(cis):
        for ci in cis:
            emit_madd(0, g0, ci)
            pi = MADD_S[0][ci][0]
            remaining[pi] -= 1
            if remaining[pi] == 0:
                emit_store(0, pi)

    for g in range(2, NG1 - 2):
        emit_transposes(1, g + 2)
        emit_exp_xe(1, g)
        emit_matmuls(1, g)
        emit_tstats(1, g - 1)
        drip(DRIP_PLAN[g - 2])
    emit_exp_xe(1, NG1 - 2)
    emit_matmuls(1, NG1 - 2)
    emit_tstats(1, NG1 - 3)
    emit_exp_xe(1, NG1 - 1)
    emit_matmuls(1, NG1 - 1)
    ji1 = emit_phase2_ji(1)       # J branch head right at J-stop
    emit_tstats(1, NG1 - 2)
    emit_tstats(1, NG1 - 1)

    stats1 = emit_phase2_stats(1)
    lt01 = emit_phase2_lt0(1, ji1)
    mib1 = emit_phase2_mi(1, stats1, ji1, lt01)
    g1, g64_1 = emit_phase2_g(1, mib1, stats1)

    # tail: PE+Act chunks and DVE chunks run in parallel; each 1024-col
    # half is stored the moment its chunks are written.  Dg = diag(g1)
    # feeds the PE path (4x tensor_scalar off ident)
    dg1 = small.tile([P, P], FP16, tag="dg", name="dg1")
    nc.vector.tensor_scalar_mul(out=dg1, in0=ident_h, scalar1=g1)
    for ci in (5, 6):                 # pe: p2[1024:2048]
        emit_madd(1, g1, ci, dg=dg1)
    emit_madd(1, g1, 4)               # dve: p2[0:1024]
    emit_store(1, 2, 0, 1024)
    emit_store(1, 2, 1024, 2048)
    for ci in (7, 8):                 # pe: p3[0:1024]
        emit_madd(1, g1, ci, dg=dg1)
    emit_madd(1, g1, 0)               # dve: p0[0:1024]
    emit_store(1, 3, 0, 1024)
    emit_store(1, 0, 0, 1024)
    for ci in (9, 10):                # pe: p3[1024:2048]
        emit_madd(1, g1, ci, dg=dg1)
    emit_madd(1, g1, 1)               # dve: p0[1024:2048]
    emit_store(1, 3, 1024, 2048)
    emit_store(1, 0, 1024, 2048)
    emit_madd(1, g1, 2)               # dve: p1[0:1024]
    emit_store(1, 1, 0, 1024)
    emit_madd(1, g1, 3)               # dve: p1[1024:2048]
    emit_store(1, 1, 1024, 2048)


_PROGRAM = None


def _get_program():
    global _PROGRAM
    if _PROGRAM is None:
        _PROGRAM = _build_program()
    return _PROGRAM


def kernel(vis_feat: np.ndarray, text_feat: np.ndarray) -> np.ndarray:
    nc = _get_program()
    vis = np.ascontiguousarray(vis_feat, dtype=np.float16)
    text = np.ascontiguousarray(text_feat, dtype=np.float16)
    bpc = B // NCORES
    in_maps = [
        {
            "vis": np.ascontiguousarray(
                vis[i * bpc:(i + 1) * bpc].reshape(bpc, C, 2, HH)
                .transpose(0, 2, 1, 3)),
            "text": np.ascontiguousarray(
                text[i * bpc:(i + 1) * bpc].reshape(bpc, C, 2, HH)
                .transpose(0, 2, 1, 3)),
        }
        for i in range(NCORES)
    ]
    res = run_bass_kernel_spmd(nc, in_maps, list(range(NCORES)))
    out = np.concatenate(
        [np.asarray(r["out"]).reshape(bpc, 2, C, HH).transpose(0, 2, 1, 3)
         .reshape(bpc, C, H, W) for r in res.results],
        axis=0)
    return out.astype(np.float32)


# revision 38
# speedup vs baseline: 1.1320x; 1.0106x over previous
"""EntropyGuidance Trainium2 kernel, fp16-I/O Act-roofline variant.

Each core handles 2 samples (B=16 over 8 cores), each sample packed as
[128 partitions = 64 channels x 2 HW-halves (p = 2c+h), 8192 free].

The Activation engine is the roofline here (~31us of exp work that no
other engine can run), so the schedule keeps Act 100% fed:
  - fp16 on both DMA directions (host casts): loads ride Pool/SWDGE
    (text) + SP/HWDGE (vis), stores SP/HWDGE; DMA busy ~35us.
  - PE is warmed with ~8 wide dummy matmuls before the first transpose
    so the p-state ramp never doubles transpose time mid-pipeline.
  - sample 0 leads with two 512-col groups so the first exp starts
    ~1us earlier; the exp->xe->transpose PSUM round-trip is kept under
    2x the exp stage time (2 tv buffers).
  - phase-2 Act ops are emitted AFTER the next sample's first exp so a
    blocked Ln never head-of-line blocks the exp stream.
  - out = vis + g*text is split DVE (tensor_scalar_mul 4x + tensor_add
    2x, 0.78 ns/col) / Pool (one-op scalar_tensor_tensor, 1.39 ns/col)
    so the tail after g1 shrinks and DVE never starves the xe products.
"""

import sys

sys.path.insert(0, "/opt/trn_rl_repo")

import math
from contextlib import ExitStack

import numpy as np

import concourse.bacc as bacc
import concourse.tile as tile
from concourse import mybir
from concourse.bass_utils import run_bass_kernel_spmd
from concourse.masks import make_identity

if not hasattr(bacc, "_orig_get_act_tables"):
    bacc._orig_get_act_tables = bacc.get_activation_tables


def _lnexp_only_tables(module_arch):
    tabs = bacc._orig_get_act_tables(module_arch)
    return {
        name: (funcs if name == "natural_log_exp_and_others" else set())
        for name, funcs in tabs.items()
    }


bacc.get_activation_tables = _lnexp_only_tables

F32 = mybir.dt.float32
FP16 = mybir.dt.float16
AF = mybir.ActivationFunctionType
ALU = mybir.AluOpType

B, C, H, W = 16, 64, 128, 128
HW = H * W                      # 16384
HH = HW // 2                    # 8192 per half
NCORES = 8
P = 128                         # partitions = 64 channels x 2 halves
EPS = 1e-9

# per-sample, per-tensor load pieces along the 8192 free axis.
# text leads with small pieces (it rides the fast SP queue for sample
# 0 and gates each group's first exp half); vis uses four 2048 pieces
# so the slower Pool prep stream still delivers 2-3 groups ahead.
TPIECES_S = [
    [(0, 512), (512, 512), (1024, 1024), (2048, 2048), (4096, 2048),
     (6144, 2048)],
    [(0, 2048), (2048, 2048), (4096, 2048), (6144, 2048)],
]
VPIECES_S = [
    [(0, 2048), (2048, 2048), (4096, 2048), (6144, 2048)],
    [(0, 2048), (2048, 2048), (4096, 2048), (6144, 2048)],
]
# out/store pieces (madd chunks must not straddle them)
PIECES_S = [
    [(0, 512), (512, 512), (1024, 1024), (2048, 2048), (4096, 2048),
     (6144, 2048)],
    [(0, 2048), (2048, 2048), (4096, 2048), (6144, 2048)],
]
# transpose/exp groups (off, width); J blocks per group = width/128.
# sample 1 ends with two 512 groups so the last exp -> J/T-stop chain
# (which gates g1 and the tail stores) is as short as possible
GROUPS_S = [
    [(0, 512), (512, 512)] + [(1024 + g * 1024, 1024) for g in range(7)],
    [(g * 1024, 1024) for g in range(7)] + [(7168, 512), (7680, 512)],
]
# madd work split: (piece, col_lo, col_hi, engine)
# DVE gets 2-op (0.78/col) chunks; sample-0 chunks are 512 wide so an
# in-flight chunk never delays an xe product by more than ~0.4us.
# "pe" chunks run Dg@text + I@vis on the Tensor engine into PSUM with
# an Act PSUM->SBUF fp16 copy (both engines idle at the tail; PE
# chunks are 512 wide since a matmul output must fit one PSUM bank)
MADD_S = [
    # sample 0: 16 x 512-col chunks; "dvepool" = DVE 4x tensor_scalar
    # for g*text then the +vis add on the idle Pool engine, so each
    # group iteration absorbs ~3 chunks without out-pacing the 1.9us
    # Act stage on DVE
    [(0, 0, 512, "dve"), (1, 0, 512, "dve"),
     (2, 0, 512, "dve"), (2, 512, 1024, "dve"),
     (3, 0, 512, "dve"), (3, 512, 1024, "dve"),
     (3, 1024, 1536, "dvepool"), (3, 1536, 2048, "dvepool"),
     (4, 0, 512, "dvepool"), (4, 512, 1024, "dvepool"),
     (4, 1024, 1536, "dvepool"), (4, 1536, 2048, "dvepool"),
     (5, 0, 512, "dvepool"), (5, 512, 1024, "dvepool"),
     (5, 1024, 1536, "dvepool"), (5, 1536, 2048, "dvepool")],
    # sample 1 (tail): DVE 5120 cols, PE+Act 3072 cols in parallel
    [(0, 0, 1024, "dve"), (0, 1024, 2048, "dve"),
     (1, 0, 1024, "dve"), (1, 1024, 2048, "dve"),
     (2, 0, 1024, "dve"),
     (2, 1024, 1536, "pe"), (2, 1536, 2048, "pe"),
     (3, 0, 512, "pe"), (3, 512, 1024, "pe"),
     (3, 1024, 1536, "pe"), (3, 1536, 2048, "pe")],
]


def _find_piece(pieces, off, w):
    for pi, (o, pw) in enumerate(pieces):
        if o <= off and off + w <= o + pw:
            return (pi, off - o)
    raise AssertionError((off, w))


def _grp_src(pieces, groups):
    return [_find_piece(pieces, off, w) for off, w in groups]


TSRC_S = [_grp_src(TPIECES_S[s], GROUPS_S[s]) for s in range(2)]
VSRC_S = [_grp_src(VPIECES_S[s], GROUPS_S[s]) for s in range(2)]
NBLK = 64                       # J blocks per sample
# Ln(joint) rescale: raw J entries are ~Sv*St/HW ~ 4.5e4, so scale the act
# Ln input to ~1.4 and add ln(HW^2 / JSCALE) back via lnSv
JSCALE = 1.0 / 32768.0
LNK0 = math.log(float(HW) * float(HW) / JSCALE)


def _build_program():
    nc = bacc.Bacc()
    vis_d = nc.declare_dram_parameter("vis", [2, 2, C, HH], FP16,
                                      isOutput=False)
    text_d = nc.declare_dram_parameter("text", [2, 2, C, HH], FP16,
                                       isOutput=False)
    out_d = nc.declare_dram_parameter("out", [2, 2, C, HH], FP16,
                                      isOutput=True)

    with ExitStack() as ctx:
        tc = ctx.enter_context(tile.TileContext(nc))
        _emit(ctx, tc, vis_d, text_d, out_d)
    nc.finalize()
    return nc


def _emit(ctx: ExitStack, tc: tile.TileContext, vis_d, text_d, out_d):
    nc = tc.nc

    io = ctx.enter_context(tc.tile_pool(name="io", bufs=2))
    etvp = ctx.enter_context(tc.tile_pool(name="etv", bufs=2))
    xep = ctx.enter_context(tc.tile_pool(name="xe", bufs=2))
    outp = ctx.enter_context(tc.tile_pool(name="outp", bufs=2))
    consts = ctx.enter_context(tc.tile_pool(name="consts", bufs=1))
    small = ctx.enter_context(tc.tile_pool(name="small", bufs=2))
    # PSUM budget (8 banks): tv 3 bufs x 2 banks = 6, plus ONE bank
    # holding both samples' J/stat accumulators AND every phase-2
    # matmul output AND the warm-up target, packed as column ranges of
    # a single [P, 512] f32 tile.  The third tv buffer gives the
    # exp->xe->transpose round-trip enough slack that the Act exp
    # stream (the roofline) never stalls on it.
    tvps = ctx.enter_context(tc.tile_pool(name="tvps", bufs=3, space="PSUM"))
    jst = ctx.enter_context(tc.tile_pool(name="jst", bufs=1, space="PSUM"))

    tsb = {}   # (s, piece) -> text fp16 tile
    vsb = {}
    jtl = {}   # s -> [P, 132] f32 PSUM: J (0:128) + S_v/S_t/T (128:131)
    tvl = {}   # (s, g) -> transpose-group PSUM tile
    etl = {}   # (s, g) -> exp(group) SBUF tile
    xel = {}   # (s, g) -> t*e^t group tile

    # fp16 identity for the 128x128 PE transposes
    ident_h = consts.tile([P, P], FP16)

    def emit_loads(s, with_ident=False):
        # ALL loads ride the single Pool/SWDGE queue: transfers then hit
        # the (exclusive) DMA device in exactly emission order
        # (t0,v0,t1,v1,...), so each transpose group's pair lands
        # together and the exp stream is paced by deliveries, never by
        # cross-queue arbitration (HWDGE queues would otherwise race
        # ahead with vis/s1 pieces and starve the s0 text stream).
        # sample-0 text pieces ride SP/HWDGE (they all land early; the
        # exp split runs the text half first) while the vis stream
        # paces with the Pool preps.  Sample 1 is entirely on Pool so
        # it can never overtake sample 0's stream at the DMA device.
        tps, vps = TPIECES_S[s], VPIECES_S[s]
        for pi, (o, w) in enumerate(tps):
            t = io.tile([P, w], FP16, tag=f"t{pi}", name=f"t{s}_{pi}")
            tsb[(s, pi)] = t
            src_t = text_d[s, :, :, o:o + w].rearrange("h c n -> (h c) n")
            (nc.sync if s == 0 else nc.gpsimd).dma_start(out=t, in_=src_t)
        for pi, (o, w) in enumerate(vps):
            v = io.tile([P, w], FP16, tag=f"v{pi}", name=f"v{s}_{pi}")
            vsb[(s, pi)] = v
            src_v = vis_d[s, :, :, o:o + w].rearrange("h c n -> (h c) n")
            nc.gpsimd.dma_start(out=v, in_=src_v)
            if with_ident and pi == 0:
                make_identity(nc, ident_h)

    def emit_transposes(s, g):
        off, w = GROUPS_S[s][g]
        tpi, tlo = TSRC_S[s][g]
        vpi, vlo = VSRC_S[s][g]
        t_src, v_src = tsb[(s, tpi)], vsb[(s, vpi)]
        nb = w // 128
        tv = tvps.tile([P, 2 * w], FP16, tag="tv", name=f"tv{s}_{g}")
        tvl[(s, g)] = tv
        for b in range(nb):
            c0 = tlo + b * 128
            nc.tensor.transpose(tv[:, b * 128:(b + 1) * 128],
                                t_src[:, c0:c0 + 128], ident_h)
        for b in range(nb):
            c0 = vlo + b * 128
            nc.tensor.transpose(tv[:, w + b * 128:w + (b + 1) * 128],
                                v_src[:, c0:c0 + 128], ident_h)

    def emit_exp_xe(s, g, split=False):
        off, w = GROUPS_S[s][g]
        tv = tvl[(s, g)]
        etv = etvp.tile([P, 2 * w], FP16, tag="etv", name=f"etv{s}_{g}")
        etl[(s, g)] = etv
        if split:
            # text half first so xe (and the J rhs) unblock earlier
            nc.scalar.activation(out=etv[:, 0:w], in_=tv[:, 0:w],
                                 func=AF.Exp)
            nc.scalar.activation(out=etv[:, w:2 * w], in_=tv[:, w:2 * w],
                                 func=AF.Exp)
        else:
            nc.scalar.activation(out=etv, in_=tv, func=AF.Exp)
        xe = xep.tile([P, w], FP16, tag="xe", name=f"xe{s}_{g}")
        xel[(s, g)] = xe
        # tensor_tensor gets the 2x DVE mode; this read also releases tv
        nc.vector.tensor_mul(xe, tv[:, 0:w], etv[:, 0:w])

    def _blk0(s, g):
        return GROUPS_S[s][g][0] // 128

    def emit_matmuls(s, g):
        off, w = GROUPS_S[s][g]
        etv, j_t = etl[(s, g)], jtl[s]
        for b in range(w // 128):
            k = _blk0(s, g) + b
            sp_f = (k == NBLK - 1)
            etT = etv[:, b * 128:(b + 1) * 128]
            evT = etv[:, w + b * 128:w + (b + 1) * 128]
            # all chains accumulate onto PSUM zeros with start=False;
            # lhsT=etT so JJ[pt, pv] rows are text-side (the d index
            # the phase-2 rst row-scale and d-contraction need)
            nc.tensor.matmul(j_t[:, 0:128], lhsT=etT, rhs=evT,
                             start=False, stop=sp_f,
                             skip_group_check=True)
            nc.tensor.matmul(j_t[:, 128:129], lhsT=evT, rhs=ones128h,
                             start=False, stop=sp_f,
                             skip_group_check=True)
            nc.tensor.matmul(j_t[:, 129:130], lhsT=etT, rhs=ones128h,
                             start=False, stop=sp_f,
                             skip_group_check=True)

    def emit_tstats(s, g):
        # T = sum_n t*e^t column sums; emitted one group late so the PE
        # queue never blocks on the DVE xe product at the queue head
        off, w = GROUPS_S[s][g]
        xe, j_t = xel[(s, g)], jtl[s]
        for b in range(w // 128):
            k = _blk0(s, g) + b
            nc.tensor.matmul(j_t[:, 130:131],
                             lhsT=xe[:, b * 128:(b + 1) * 128],
                             rhs=ones128h, start=False,
                             stop=(k == NBLK - 1),
                             skip_group_check=True)

    def emit_phase2_stats(s):
        """negent/recips branch; independent of the J merge. Reads the
        Sv/St/T stat columns straight out of PSUM where possible."""
        # h-fold via a partition-strided DVE add straight out of PSUM:
        # sums[c] = stats[2c] + stats[2c+1], one op, no PE round-trip
        sh = small.tile([C, 3], F32, tag="sumh", name=f"sumh{s}")
        nc.vector.tensor_copy(out=sh, in_=jtl[s][C:P, 128:131])
        sums = small.tile([C, 3], F32, tag="sums", name=f"sums{s}")
        nc.vector.tensor_add(sums, sh, jtl[s][0:C, 128:131])
        recips = small.tile([C, 3], F32, tag="recips", name=f"recips{s}")
        nc.vector.reciprocal(out=recips[:, 0:2], in_=sums[:, 0:2])
        rst = recips[:, 1:2]
        nc.vector.tensor_scalar_mul(out=recips[:, 2:3], in0=recips[:, 0:1],
                                    scalar1=0.5)
        lnls = small.tile([C, 2], F32, tag="lnls", name=f"lnls{s}")
        nc.scalar.activation(out=lnls, in_=sums[:, 0:2], func=AF.Ln)
        # fold the lt0 rescale constant ln(HW^2 / JSCALE) into lnSv so the
        # Ln of the raw joint runs on O(1) inputs (act table accuracy)
        nc.vector.tensor_scalar_add(out=lnls[:, 0:1], in0=lnls[:, 0:1],
                                    scalar1=-LNK0)
        # negent = T/St - lnSt; bias for the final Exp folds negent in:
        # biasv = -negent - (1 + HW*EPS)
        negent = small.tile([C, 1], F32, tag="negent", name=f"negent{s}")
        nc.vector.scalar_tensor_tensor(
            out=negent, in0=sums[:, 2:3], scalar=rst, in1=lnls[:, 1:2],
            op0=ALU.mult, op1=ALU.subtract)
        biasv = small.tile([C, 1], F32, tag="biasv", name=f"biasv{s}")
        nc.vector.tensor_scalar(out=biasv, in0=negent, scalar1=-1.0,
                                scalar2=-(1.0 + HW * EPS), op0=ALU.mult,
                                op1=ALU.add)
        r2 = small.tile([C, 2], F32, tag="r2", name=f"r2{s}")
        nc.vector.memset(r2[:, 0:1], 1.0)
        nc.vector.tensor_copy(out=r2[:, 1:2], in_=lnls[:, 1:2])
        return recips, lnls, biasv, r2

    def emit_phase2_ji(s):
        """J-branch head: only needs the J matmul chain stop (emitted
        before the T-stat stop so it starts ~0.5us earlier).
        J64T[d,c] = sum_h JJ[(c,h),(d,h)]"""
        jh = small.tile([C, C], F32, tag="jjh", name=f"jjh{s}")
        nc.vector.tensor_copy(out=jh, in_=jtl[s][C:P, C:P])
        j64sum = small.tile([C, C], F32, tag="jj", name=f"jj{s}")
        nc.vector.tensor_add(j64sum, jh, jtl[s][0:C, 0:C])
        return j64sum

    def emit_phase2_lt0(s, j64sum):
        lt0 = small.tile([C, C], F32, tag="lt0", name=f"lt0{s}")
        nc.scalar.activation(out=lt0, in_=j64sum, func=AF.Ln,
                             scale=JSCALE)
        return lt0

    def emit_phase2_mi(s, stats, j64sum, lt0):
        """mi via the separable log:
        ln(HW^2*J/(Sv*St)) = ln(HW^2*Jraw) - lnSv_c - lnSt_d
        (the +EPS inside the reference log shifts values ~1e-9; dropped)
        """
        recips, lnls, biasv, r2 = stats
        rsv05 = recips[:, 2:3]
        rst = recips[:, 1:2]
        lnsv = lnls[:, 0:1]
        # PSUM->SBUF copy doubles as the rst_d row scaling
        j64t = small.tile([C, C], F32, tag="j64t_sb", name=f"j64t_sb{s}")
        nc.vector.tensor_scalar_mul(out=j64t, in0=j64sum, scalar1=rst)
        q = small.tile([C, C], F32, tag="q", name=f"q{s}")
        nc.vector.tensor_mul(q, lt0, j64t)
        ry_ps = jt_all[0:C, 459:461]
        nc.tensor.matmul(ry_ps, lhsT=j64t, rhs=r2, start=True, stop=True)
        u1_ps = jt_all[0:C, 461:462]
        nc.tensor.matmul(u1_ps, lhsT=q, rhs=ones64[:, 0:1], start=True,
                         stop=True)
        # z1 = ry0*lnsv + ry1 ; zz = (u1 - z1) * 0.5/Sv  (two-scalar ops)
        z1 = small.tile([C, 1], F32, tag="z1", name=f"z1{s}")
        nc.vector.tensor_scalar(out=z1, in0=ry_ps[:, 0:1], scalar1=lnsv,
                                scalar2=ry_ps[:, 1:2], op0=ALU.mult,
                                op1=ALU.add)
        zz = small.tile([C, 1], F32, tag="zz", name=f"zz{s}")
        nc.vector.tensor_scalar(out=zz, in0=u1_ps, scalar1=z1,
                                scalar2=rsv05, op0=ALU.subtract,
                                op1=ALU.mult)
        mib_ps = jt_all[0:C, 462:463]
        nc.tensor.matmul(mib_ps, lhsT=ones64, rhs=zz, start=True,
                         stop=True)
        return mib_ps

    def emit_phase2_g(s, mib_ps, stats):
        biasv = stats[2]
        g64 = small.tile([C, 1], F32, tag="g64", name=f"g64{s}")
        nc.scalar.activation(out=g64, in_=mib_ps, func=AF.Exp, scale=-1.0,
                             bias=biasv)
        nc.vector.tensor_scalar_add(out=g64, in0=g64, scalar1=1.0)
        nc.vector.reciprocal(out=g64, in_=g64)
        g = small.tile([P, 1], F32, tag="g", name=f"g{s}")
        nc.vector.tensor_copy(out=g[0:C, :], in_=g64)
        nc.vector.tensor_copy(out=g[C:P, :], in_=g64)
        return g, g64

    otl = {}

    def emit_madd(s, g, ci, dg=None):
        pi, lo, hi, eng = MADD_S[s][ci]
        o, w = PIECES_S[s][pi]
        if (s, pi) not in otl:
            otl[(s, pi)] = outp.tile([P, w], FP16, tag=f"o{s}_{pi}",
                                     name=f"o{s}_{pi}")
        ot = otl[(s, pi)]
        tpi, tlo = _find_piece(TPIECES_S[s], o + lo, hi - lo)
        vpi, vlo = _find_piece(VPIECES_S[s], o + lo, hi - lo)
        tin = tsb[(s, tpi)][:, tlo:tlo + hi - lo]
        vin = vsb[(s, vpi)][:, vlo:vlo + hi - lo]
        if eng == "dve" or eng == "dvepool":
            # g*text at 4x (f32 ptr scalar exempt from the 2-byte rule),
            # then += vis at 2x (or on the idle Pool for dvepool)
            nc.vector.tensor_scalar_mul(out=ot[:, lo:hi], in0=tin,
                                        scalar1=g)
            add_eng = nc.vector if eng == "dve" else nc.gpsimd
            add_eng.tensor_add(ot[:, lo:hi], ot[:, lo:hi], vin)
        else:
            # Dg@text + I@vis accumulated in PSUM, Act copies to fp16
            pm = tvps.tile([P, hi - lo], F32, tag="tv",
                           name=f"pm{s}_{ci}")
            nc.tensor.matmul(pm, lhsT=dg, rhs=tin,
                             start=True, stop=False)
            nc.tensor.matmul(pm, lhsT=ident_h, rhs=vin,
                             start=False, stop=True)
            nc.scalar.activation(out=ot[:, lo:hi], in_=pm, func=AF.Copy)

    store_ctr = [0]

    def emit_store(s, pi, lo=None, hi=None):
        # alternate SP/Pool queues so store issue never serializes on
        # one sequencer at the tail; lo/hi store a piece sub-range so
        # tail halves stream out as soon as their chunks finish
        o, w = PIECES_S[s][pi]
        if lo is None:
            lo, hi = 0, w
        dst = out_d[s, :, :, o + lo:o + hi].rearrange("h c n -> (h c) n")
        # sample-0 stores always ride SP: the Pool queue is busy with
        # sample-1 load preps and the Pool-assisted madd adds mid-kernel
        if s == 0:
            eng = nc.sync
        else:
            eng = nc.sync if store_ctr[0] % 2 == 0 else nc.gpsimd
            store_ctr[0] += 1
        eng.dma_start(out=dst, in_=otl[(s, pi)][:, lo:hi])

    # ---- emission ----
    # single-bank PSUM mega-tile: j0 0:132, j1 132:264, warm 264:392,
    # j64t 392:456, sums 456:459, ry 459:461, u1 461, mib 462, gbc 463
    jt_all = jst.tile([P, 512], F32, tag="jall", name="jall")
    for s in range(2):
        jtl[s] = jt_all[:, s * 132:(s + 1) * 132]

    # DVE-built constants + PE warm-up fodder (DVE is idle at t=0)
    ones128h = consts.tile([P, 1], FP16)
    nc.vector.memset(ones128h, 1.0)
    junk = consts.tile([P, 192], FP16)
    nc.vector.memset(junk, 0.0)

    emit_loads(0, with_ident=True)

    # nudge the PE p-state ramp before the first transposes (engine
    # init means PE can't start before ~2.4us; the first piece lands
    # ~2.9us, so just a few warms to leave the lowest p-state)
    warm_ps = jt_all[0:1, 264:392]
    for i in range(3):
        nc.tensor.matmul(warm_ps, lhsT=ones128h, rhs=junk[:, 0:128],
                         start=True, stop=True)

    ones64 = consts.tile([C, C], F32)
    nc.gpsimd.memset(ones64, 1.0)

    NG0 = len(GROUPS_S[0])
    NG1 = len(GROUPS_S[1])

    # sample 0, software-pipelined: transposes TWO groups ahead of the
    # exp (3 tv buffers), so group g's J matmuls waiting on exp(g) at
    # the PE queue head never block the transposes of group g+2;
    # T-stat matmuls one group behind (they wait on DVE xe)
    emit_transposes(0, 0)
    emit_transposes(0, 1)
    emit_transposes(0, 2)
    emit_exp_xe(0, 0, split=True)
    emit_matmuls(0, 0)
    for g in range(1, NG0 - 2):
        emit_transposes(0, g + 2)
        emit_exp_xe(0, g)
        emit_matmuls(0, g)
        emit_tstats(0, g - 1)
    emit_exp_xe(0, NG0 - 2)
    emit_matmuls(0, NG0 - 2)
    emit_tstats(0, NG0 - 3)
    emit_exp_xe(0, NG0 - 1)
    emit_matmuls(0, NG0 - 1)
    ji0 = emit_phase2_ji(0)       # J branch head right at J-stop
    emit_tstats(0, NG0 - 2)
    emit_tstats(0, NG0 - 1)

    emit_loads(1)

    # sample 1 pipelined; sample-0 phase 2 is emitted AFTER exp(1,0) so
    # its (dependency-blocked) Act ops never head-of-line block the exp
    # stream; its Act ops then slot between sample-1 exps
    emit_transposes(1, 0)
    emit_transposes(1, 1)
    emit_transposes(1, 2)
    emit_exp_xe(1, 0)
    emit_matmuls(1, 0)

    stats0 = emit_phase2_stats(0)
    lt00 = emit_phase2_lt0(0, ji0)
    mib0 = emit_phase2_mi(0, stats0, ji0, lt00)

    emit_transposes(1, 3)
    emit_exp_xe(1, 1)
    emit_matmuls(1, 1)
    emit_tstats(1, 0)

    g0, g64_0 = emit_phase2_g(0, mib0, stats0)

    # s0 madd chunks drip into the group iterations AFTER the xe
    # product; Pool-assisted chunks go first (their Pool adds must
    # clear the Pool queue before phase-2(1) needs it), and the short
    # 512-group iterations at the tail carry at most one chunk
    remaining = {}
    for ci, (pi, lo, hi, eng) in enumerate(MADD_S[0]):
        remaining[pi] = remaining.get(pi, 0) + 1
    DRIP_PLAN = [[8, 9, 10, 14], [11, 12, 13, 15], [6, 7, 0], [1, 2, 3], [4, 5]]

    def drip(cis):
        for ci in cis:
            emit_madd(0, g0, ci)
            pi = MADD_S[0][ci][0]
            remaining[pi] -= 1
            if remaining[pi] == 0:
                emit_store(0, pi)

    for g in range(2, NG1 - 2):
        emit_transposes(1, g + 2)
        emit_exp_xe(1, g)
        emit_matmuls(1, g)
        emit_tstats(1, g - 1)
        drip(DRIP_PLAN[g - 2])
    emit_exp_xe(1, NG1 - 2)
    emit_matmuls(1, NG1 - 2)
    emit_tstats(1, NG1 - 3)
    emit_exp_xe(1, NG1 - 1)
    emit_matmuls(1, NG1 - 1)
    ji1 = emit_phase2_ji(1)       # J branch head right at J-stop
    emit_tstats(1, NG1 - 2)
    emit_tstats(1, NG1 - 1)

    stats1 = emit_phase2_stats(1)
    lt01 = emit_phase2_lt0(1, ji1)
    mib1 = emit_phase2_mi(1, stats1, ji1, lt01)
    g1, g64_1 = emit_phase2_g(1, mib1, stats1)

    # tail: PE+Act chunks and DVE chunks run in parallel; each 1024-col
    # half is stored the moment its chunks are written.  Dg = diag(g1)
    # feeds the PE path (4x tensor_scalar off ident)
    dg1 = small.tile([P, P], FP16, tag="dg", name="dg1")
    nc.vector.tensor_scalar_mul(out=dg1, in0=ident_h, scalar1=g1)
    for ci in (5, 6):                 # pe: p2[1024:2048]
        emit_madd(1, g1, ci, dg=dg1)
    emit_madd(1, g1, 4)               # dve: p2[0:1024]
    emit_store(1, 2, 0, 1024)
    emit_store(1, 2, 1024, 2048)
    for ci in (7, 8):                 # pe: p3[0:1024]
        emit_madd(1, g1, ci, dg=dg1)
    emit_madd(1, g1, 0)               # dve: p0[0:1024]
    emit_store(1, 3, 0, 1024)
    emit_store(1, 0, 0, 1024)
    for ci in (9, 10):                # pe: p3[1024:2048]
        emit_madd(1, g1, ci, dg=dg1)
    emit_madd(1, g1, 1)               # dve: p0[1024:2048]
    emit_store(1, 3, 1024, 2048)
    emit_store(1, 0, 1024, 2048)
    emit_madd(1, g1, 2)               # dve: p1[0:1024]
    emit_store(1, 1, 0, 1024)
    emit_madd(1, g1, 3)               # dve: p1[1024:2048]
    emit_store(1, 1, 1024, 2048)


_PROGRAM = None


def _get_program():
    global _PROGRAM
    if _PROGRAM is None:
        _PROGRAM = _build_program()
    return _PROGRAM


def kernel(vis_feat: np.ndarray, text_feat: np.ndarray) -> np.ndarray:
    nc = _get_program()
    vis = np.ascontiguousarray(vis_feat, dtype=np.float16)
    text = np.ascontiguousarray(text_feat, dtype=np.float16)
    bpc = B // NCORES
    in_maps = [
        {
            "vis": np.ascontiguousarray(
                vis[i * bpc:(i + 1) * bpc].reshape(bpc, C, 2, HH)
                .transpose(0, 2, 1, 3)),
            "text": np.ascontiguousarray(
                text[i * bpc:(i + 1) * bpc].reshape(bpc, C, 2, HH)
                .transpose(0, 2, 1, 3)),
        }
        for i in range(NCORES)
    ]
    res = run_bass_kernel_spmd(nc, in_maps, list(range(NCORES)))
    out = np.concatenate(
        [np.asarray(r["out"]).reshape(bpc, 2, C, HH).transpose(0, 2, 1, 3)
         .reshape(bpc, C, H, W) for r in res.results],
        axis=0)
    return out.astype(np.float32)
